# revision 1
# baseline (speedup 1.0000x reference)
"""Trainium2 Bass kernel for nn_Attention_60739427500161.

Strategy (8 NeuronCores, one chip, no collectives):
- Sequence-sharded (context parallel): core c handles batch b=c//4 and two
  zigzag 256-row query strips (ci*256 and (7-ci)*256, ci=c%4) so causal work
  is balanced. Each core computes q/gate for its 512 rows, the full-batch k
  projection locally, runs attention + gating + out_proj for its rows, and
  writes its 512 output rows. The host scatters them back.
- All matmuls run in bf16. Scores are computed transposed (scoresT[j,i]) so
  softmax needs no PE transposes; denominator l via a ones-column in the av
  matmul; gating computes av / (l * (1 + e^{-g})).
- RoPE rotate_half is a host-side feature permutation so the partner lives
  one partition away and a DVE stream_shuffle([o^1]) produces the rotated
  operand.
- Phase layout: [warmup: 12 qg tiles while wk/xk DMA streams on the scalar
  queue] -> [kproj with transposes interleaved at tf boundaries] -> [16
  attention pair-steps with qg-filler tiles] -> [outproj, wo prefetched
  during attention]. Within a pair, the two heads' 64-contraction score
  matmuls are emitted alternating A/B so each LDWEIGHTS targets the row
  tile ((0,0) vs (64,0)) opposite the in-flight matmul and pulls ahead:
  measured sc matmuls drop to ~stream rate (~119ns for N=256).
- Hard-won scheduling facts baked in here: gpsimd ucode libraries
  (affine_select + partition_broadcast) are preloaded with dummy ops at
  t=0 (first use otherwise costs a ~7us LIBRARY_RELOAD stall mid-pairs);
  transpose-mode does NOT count as PE-busy for the HAM clock governor, so
  transposes must be surrounded by dense matmuls or the PE re-throttles to
  1.2GHz; late pairs keep 1 filler qg tile each so the PE never idles a
  3.4us HAM window before outproj; av psums use 3 bufs so the next pair's
  av chain never waits on the previous pair's gating DVE chain. PSUM
  budget is exactly 8 banks: sc 2x[128,1024] + av 3x[65,512] + qg 1.
- wqp/xTq are host-pre-laid-out so every wq/xq DMA is a dense
  4-16KB-per-partition transfer: the previous (kc p) m gather pattern read
  256B HBM segments at 32KB stride, throttling the DMA-bound warmup window
  (dense layout moved the settled time 484 -> 476us).
- Chip-level P0 power-state variance is real: identical binaries measure
  +/-9% run to run (2.0 vs 2.4GHz PE clock under sustained load).
"""

import sys

for _p in ("/root/.axon_site/_ro/trn_rl_repo", "/opt/trn_rl_repo"):
    if _p not in sys.path:
        sys.path.append(_p)

import ml_dtypes
import numpy as np

import concourse.bass as bass
import concourse.mybir as mybir
import concourse.tile as tile
from concourse import bacc
from concourse.bass_utils import run_bass_kernel_spmd
from concourse.masks import make_identity

F32 = mybir.dt.float32
BF16 = mybir.dt.bfloat16
AF = mybir.ActivationFunctionType
ALU = mybir.AluOpType

B, S, HID = 2, 2048, 2048
NH, NKV, D = 32, 8, 64

# pi permutation: interleave (d, d+32) pairs so rotate_half partner is the
# adjacent partition. pos(d) = 2d (d<32) else 2(d-32)+1.
_POS = np.array([2 * d if d < 32 else 2 * (d - 32) + 1 for d in range(D)])
_INV = np.argsort(_POS)
_SHUF_MASK = [o ^ 1 for o in range(32)]

# q-head placement: head h must share its SBUF row base (0 or 64) with its
# kv head hk=h//4. Tile t pairs one even-hk head (rows 0-63) with one odd-hk
# head (rows 64-127); both heads of tile t form attention pair t.
_EVENS = [h for h in range(NH) if (h // 4) % 2 == 0]
_ODDS = [h for h in range(NH) if (h // 4) % 2 == 1]
_QTILE = [0] * NH
_QROW = [0] * NH
for _i, _h in enumerate(_EVENS):
    _QTILE[_h], _QROW[_h] = _i, 0
for _i, _h in enumerate(_ODDS):
    _QTILE[_h], _QROW[_h] = _i, 64
_QCOL_ORDER = np.concatenate(
    [np.concatenate([_EVENS[t] * D + _INV, _ODDS[t] * D + _INV]) for t in range(16)]
)

_NC_CACHE = None
DEBUG_DUMPS = False
N_WARMUP = 6  # qg tile-pairs (q+gate) emitted before kproj


def _build_nc():
    nc = bacc.Bacc(None, target_bir_lowering=False, enable_partition_id=True)

    xTq = nc.dram_tensor("xTq", [128, 16 * 512], BF16, kind="ExternalInput")
    xTk = nc.dram_tensor("xTk", [HID, S], BF16, kind="ExternalInput")
    wqp = nc.dram_tensor("wqp", [32 * 128, 16 * 128], BF16, kind="ExternalInput")
    wkp = nc.dram_tensor("wkp", [HID, 512], BF16, kind="ExternalInput")
    wop = nc.dram_tensor("wop", [HID, HID], BF16, kind="ExternalInput")
    cosq = nc.dram_tensor("cosq", [128, 512], F32, kind="ExternalInput")
    sinq = nc.dram_tensor("sinq", [128, 512], F32, kind="ExternalInput")
    cosk = nc.dram_tensor("cosk", [128, 2048], BF16, kind="ExternalInput")
    sink = nc.dram_tensor("sink", [128, 2048], BF16, kind="ExternalInput")
    out = nc.dram_tensor("out", [512, HID], F32, kind="ExternalOutput")

    if DEBUG_DUMPS:
        dbg_qT = nc.dram_tensor("dbg_qT", [128, 8192], BF16, kind="ExternalOutput")
        dbg_sigT = nc.dram_tensor("dbg_sigT", [128, 8192], BF16, kind="ExternalOutput")
        dbg_kT = nc.dram_tensor("dbg_kT", [128, 8192], BF16, kind="ExternalOutput")
        dbg_kaug = nc.dram_tensor("dbg_kaug", [128, NKV * 16 * 65], BF16, kind="ExternalOutput")
        dbg_gatedT = nc.dram_tensor("dbg_gatedT", [128, 8192], BF16, kind="ExternalOutput")

    with tile.TileContext(nc) as tc:
        ci = nc.partition_id() % 4

        with tc.tile_pool(name="persist", bufs=1) as pers:
            qT = pers.tile([128, 16 * 512], BF16, tag="qT")
            sigT = pers.tile([128, 16 * 512], BF16, tag="sigT")
            kT = pers.tile([128, 4 * 2048], BF16, tag="kT")
            kaug = pers.tile([128, NKV * 16 * 65], BF16, tag="kaug")
            kaug4 = kaug[:].rearrange("p (h j d) -> p h j d", h=NKV, j=16)

            ident = pers.tile([128, 64], BF16, tag="ident")
            make_identity(nc, ident[0:64, :])
            nc.sync.dma_start(ident[64:128, :], ident[0:64, :])

            # Load both gpsimd ucode libraries (affine_select +
            # partition_broadcast) NOW, under the DMA lead-in shadow: the
            # first use of each costs a ~7us LIBRARY_RELOAD stall.
            gpw = pers.tile([2, 64], F32, tag="gpw")
            nc.vector.memset(gpw[:], 0.0)
            nc.gpsimd.affine_select(
                gpw[0:1, :], gpw[0:1, :], pattern=[[1, 64]],
                compare_op=ALU.is_ge, fill=0.0, base=0, channel_multiplier=-1,
            )
            nc.gpsimd.partition_broadcast(gpw[:], gpw[0:1, :])

            with (
                tc.tile_pool(name="pXq", bufs=1) as pXq,
                tc.tile_pool(name="pWq", bufs=2) as pWq,
                tc.tile_pool(name="pRt", bufs=1) as pRt,
                tc.tile_pool(name="PSqg", bufs=1, space="PSUM") as PSqg,
            ):
                # ---- warmup-stream DMAs (sync queue) ----
                # First wq tile ahead of the xq bulk so the first qg matmul
                # only waits for wq_t0 + xq chunk 0; xq split per-chunk so
                # the kc-accumulation paces with chunk arrival.
                xq = pXq.tile([128, 16 * 512], BF16, tag="xq")

                def load_wq(t):
                    # host pre-layout (t, p, kc, m): one dense 4KB/partition
                    # transfer (the old (kc p) m gather read 256B segments)
                    wq_t = pWq.tile([128, 16 * 128], BF16, tag="wq", bufs=3, name="wq_t")
                    nc.sync.dma_start(wq_t[:], wqp[t * 128 : (t + 1) * 128, :])
                    return wq_t

                wq_pre = {0: load_wq(0)}
                nc.sync.dma_start(xq[:, 0 : 2 * 512], xTq[:, 0 : 2 * 512])
                wq_pre[16] = load_wq(16)
                nc.sync.dma_start(xq[:, 2 * 512 : 8 * 512], xTq[:, 2 * 512 : 8 * 512])
                cosq_sb = pXq.tile([128, 512], F32, tag="cosq")
                sinq_sb = pXq.tile([128, 512], F32, tag="sinq")
                nc.sync.dma_start(cosq_sb[:], cosq[:])
                nc.sync.dma_start(sinq_sb[:], sinq[:])
                nc.sync.dma_start(xq[:, 8 * 512 :], xTq[:, 8 * 512 :])

                def emit_qg_tile(t, ps_pool=None, ps_bufs=1):
                    """qg projection m-tile t (q-tile if t<16 else gate)."""
                    wq_t = wq_pre.pop(t) if t in wq_pre else load_wq(t)
                    qg_ps = (ps_pool or PSqg).tile(
                        [128, 512], F32, tag="qg", bufs=ps_bufs, name="qg_ps"
                    )
                    for kc in range(16):
                        nc.tensor.matmul(
                            qg_ps[:],
                            wq_t[:, kc * 128 : (kc + 1) * 128],
                            xq[:, kc * 512 : (kc + 1) * 512],
                            start=(kc == 0),
                            stop=(kc == 15),
                        )
                    if t < 16:
                        shf = pRt.tile([128, 512], F32, tag="shf", name="shf")
                        nc.vector.stream_shuffle(shf[:], qg_ps[:], _SHUF_MASK)
                        t1 = pRt.tile([128, 512], F32, tag="t1", name="t1")
                        nc.vector.tensor_tensor(t1[:], qg_ps[:], cosq_sb[:], ALU.mult)
                        t2 = pRt.tile([128, 512], F32, tag="t2", name="t2")
                        nc.vector.tensor_tensor(t2[:], shf[:], sinq_sb[:], ALU.mult)
                        nc.vector.tensor_tensor(
                            qT[:, t * 512 : (t + 1) * 512], t1[:], t2[:], ALU.add
                        )
                    else:
                        # e^{-g}; 1/(1+e^{-g}) is folded into the gating recip
                        nc.scalar.activation(
                            sigT[:, (t - 16) * 512 : (t - 15) * 512],
                            qg_ps[:],
                            AF.Exp,
                            scale=-1.0,
                        )

                # ---- phase A DMAs on the SCALAR queue so the warmup-stream
                # wq tile DMAs (pool-gated) can't head-of-line block them ----
                with (
                    tc.tile_pool(name="pAtab", bufs=1) as pAtab,
                    tc.tile_pool(name="pA", bufs=1) as pA,
                ):
                    wk_all = pA.tile([128, 16 * 512], BF16, tag="wk")
                    xk_all = pA.tile([128, 16 * 2048], BF16, tag="xk")
                    for kh in range(16):
                        nc.scalar.dma_start(
                            wk_all[:, kh * 512 : (kh + 1) * 512],
                            wkp[kh * 128 : (kh + 1) * 128, :],
                        )
                        nc.scalar.dma_start(
                            xk_all[:, kh * 2048 : (kh + 1) * 2048],
                            xTk[kh * 128 : (kh + 1) * 128, :],
                        )
                    cosk_sb = pAtab.tile([128, 2048], BF16, tag="cosk")
                    sink_sb = pAtab.tile([128, 2048], BF16, tag="sink")
                    nc.scalar.dma_start(cosk_sb[:], cosk[:])
                    nc.scalar.dma_start(sink_sb[:], sink[:])

                    # ---- warmup: qg tiles run while wk/xk stream in.
                    # Separate 2-buf psum pool: banks are free before the
                    # attention pools open, and bufs=1 would serialize each
                    # tile on the previous tile's RoPE reads. ----
                    with tc.tile_pool(name="PSwarm", bufs=1, space="PSUM") as PSwarm:
                        for w in range(N_WARMUP):
                            emit_qg_tile(w, PSwarm, 2)
                            emit_qg_tile(16 + w, PSwarm, 2)

                    # ---- kproj (dense: xk fully resident by now) ----
                    with (
                        tc.tile_pool(name="pAr", bufs=2) as pAr,
                        tc.tile_pool(name="psA", bufs=1, space="PSUM") as psA,
                    ):
                        def emit_transposes(hk):
                            """kaug chunks for kv head hk from the RoPE'd kT.
                            Interleaved at tf boundaries so dense kproj MMs
                            surround the transpose-mode stretches (HAM does
                            not count transpose-mode as PE-busy)."""
                            hkr = (hk % 2) * 64
                            base = (hk // 2) * 2048
                            for jj in range(4):
                                tr = psA.tile([128, 256], BF16, tag="tr", bufs=2, name="tr")
                                for u in range(4):
                                    jc = jj * 4 + u
                                    nc.tensor.transpose(
                                        tr[:, u * 64 : (u + 1) * 64],
                                        kT[hkr : hkr + 64, base + jc * 128 : base + (jc + 1) * 128],
                                        ident[hkr : hkr + 64, :],
                                    )
                                nc.vector.tensor_copy(
                                    kaug4[:, hk, jj * 4 : (jj + 1) * 4, 0:64],
                                    tr[:].rearrange("p (u d) -> p u d", u=4),
                                )

                        for tf in range(4):
                            for kb in range(4):
                                kp_ps = psA.tile(
                                    [128, 512], F32, tag="kp", bufs=2, name="kp_ps"
                                )
                                for kc in range(16):
                                    nc.tensor.matmul(
                                        kp_ps[:],
                                        wk_all[:, kc * 512 + tf * 128 : kc * 512 + (tf + 1) * 128],
                                        xk_all[:, kc * 2048 + kb * 512 : kc * 2048 + (kb + 1) * 512],
                                        start=(kc == 0),
                                        stop=(kc == 15),
                                    )
                                shf = pAr.tile([128, 512], F32, tag="shf")
                                nc.vector.stream_shuffle(shf[:], kp_ps[:], _SHUF_MASK)
                                t1 = pAr.tile([128, 512], F32, tag="t1")
                                nc.vector.tensor_tensor(
                                    t1[:], kp_ps[:], cosk_sb[:, kb * 512 : (kb + 1) * 512], ALU.mult
                                )
                                t2 = pAr.tile([128, 512], F32, tag="t2")
                                nc.vector.tensor_tensor(
                                    t2[:], shf[:], sink_sb[:, kb * 512 : (kb + 1) * 512], ALU.mult
                                )
                                nc.vector.tensor_tensor(
                                    kT[:, tf * 2048 + kb * 512 : tf * 2048 + (kb + 1) * 512],
                                    t1[:],
                                    t2[:],
                                    ALU.add,
                                )
                            if tf >= 1:
                                emit_transposes(2 * (tf - 1))
                                emit_transposes(2 * (tf - 1) + 1)
                        emit_transposes(6)
                        emit_transposes(7)
                nc.vector.memset(kaug4[:, :, :, 64:65], 1.0)

                # ---- attention pair-steps ----
                with (
                    tc.tile_pool(name="pG", bufs=1) as pG,
                    tc.tile_pool(name="pO", bufs=3) as pO,
                ):
                    gatedT = pG.tile([128, 16 * 512], BF16, tag="gatedT")

                    # prefetch outproj wo chunks for oc=0 during attention
                    # (scalar DMA queue is idle now)
                    wo_ts = {}
                    def load_wo(oc):
                        lst = []
                        for fc in range(16):
                            wo_t = pO.tile([128, 512], BF16, tag="wo", bufs=18, name="wo_t")
                            nc.scalar.dma_start(
                                wo_t[:],
                                wop[fc * 128 : (fc + 1) * 128, oc * 512 : (oc + 1) * 512],
                            )
                            lst.append(wo_t)
                        wo_ts[oc] = lst
                    load_wo(0)

                    with (
                        tc.tile_pool(name="pET", bufs=6) as pET,
                        tc.tile_pool(name="pSm", bufs=3) as pSm,
                        tc.tile_pool(name="PSsc", bufs=1, space="PSUM") as PSsc,
                        tc.tile_pool(name="PSav", bufs=1, space="PSUM") as PSav,
                    ):
                        def sc_pair_group(kbase, jcs, w, rhs_A, rhs_B):
                          """Score matmuls + exp for one chunk group of BOTH
                          heads, MMs interleaved A/B so consecutive PE instrs
                          target opposite row-tiles (0,0)/(64,0) and their
                          LDWEIGHTS can pull ahead past the other tile's
                          in-flight matmul."""
                          scA = PSsc.tile([128, 1024], F32, tag="sc", bufs=2, name="scA")
                          scB = PSsc.tile([128, 1024], F32, tag="sc", bufs=2, name="scB")
                          for u, jc in enumerate(jcs):
                              nc.tensor.matmul(
                                  scA[:, u * w : u * w + w],
                                  kT[0:64, kbase + jc * 128 : kbase + (jc + 1) * 128],
                                  rhs_A,
                                  start=True,
                                  stop=True,
                              )
                              nc.tensor.matmul(
                                  scB[:, u * w : u * w + w],
                                  kT[64:128, kbase + jc * 128 : kbase + (jc + 1) * 128],
                                  rhs_B,
                                  start=True,
                                  stop=True,
                              )
                          n_tot = len(jcs) * w
                          etA = pET.tile([128, 1024], BF16, tag="et", name="etA")
                          nc.scalar.activation(etA[:, 0:n_tot], scA[:, 0:n_tot], AF.Exp)
                          etB = pET.tile([128, 1024], BF16, tag="et", name="etB")
                          nc.scalar.activation(etB[:, 0:n_tot], scB[:, 0:n_tot], AF.Exp)
                          return etA, etB

                        def masks(et, jcs, w, nja, njb):
                            for u, jc in enumerate(jcs):
                                if jc >= nja - 2 and jc < nja and w == 512:
                                    nc.gpsimd.affine_select(
                                        et[:, u * w : u * w + 256],
                                        et[:, u * w : u * w + 256],
                                        pattern=[[1, 256]],
                                        compare_op=ALU.is_ge,
                                        fill=0.0,
                                        base=(0 if jc == nja - 2 else -128),
                                        channel_multiplier=-1,
                                    )
                                if jc >= njb - 2:
                                    off = u * w + (256 if w == 512 else 0)
                                    nc.gpsimd.affine_select(
                                        et[:, off : off + 256],
                                        et[:, off : off + 256],
                                        pattern=[[1, 256]],
                                        compare_op=ALU.is_ge,
                                        fill=0.0,
                                        base=(0 if jc == njb - 2 else -128),
                                        channel_multiplier=-1,
                                    )

                        def av_group(av_ps, hk, jcs, w, et, njb):
                            for u, jc in enumerate(jcs):
                                nc.tensor.matmul(
                                    av_ps[0:65, (0 if w == 512 else 256) : 512],
                                    kaug[:, (hk * 16 + jc) * 65 : (hk * 16 + jc) * 65 + 65],
                                    et[:, u * w : u * w + w],
                                    start=(jc == 0),
                                    stop=(jc == njb - 1),
                                    skip_group_check=True,
                                )

                        def emit_gating(h, av_ps):
                            # gated = av / (l * (1 + e^{-g}))
                            tq, hr = _QTILE[h], _QROW[h]
                            lrow = pSm.tile([1, 512], F32, tag="lrow", name="lrow")
                            nc.vector.tensor_copy(lrow[:], av_ps[64:65, :])
                            lb = pSm.tile([64, 512], F32, tag="lb", name="lb")
                            nc.gpsimd.partition_broadcast(lb[:], lrow[:])
                            eg = sigT[hr : hr + 64, tq * 512 : (tq + 1) * 512]
                            if hr:
                                egc = pSm.tile([64, 512], BF16, tag="egc", name="egc")
                                nc.vector.tensor_copy(egc[:], eg)
                                eg = egc[:]
                            den = pSm.tile([64, 512], F32, tag="den", name="den")
                            nc.vector.scalar_tensor_tensor(
                                den[:], eg, 1.0, lb[:], ALU.add, ALU.mult
                            )
                            rden = pSm.tile([64, 512], F32, tag="rden", name="rden")
                            nc.vector.reciprocal_approx_fast(rden[:], den[:])
                            nc.vector.tensor_tensor(
                                gatedT[hr : hr + 64, tq * 512 : (tq + 1) * 512],
                                av_ps[0:64, :],
                                rden[:],
                                ALU.mult,
                            )

                        for arm in tc.Switch(ci, 4):
                            nja, njb = 2 * arm + 2, 16 - 2 * arm
                            groups = []
                            for g0 in range(0, nja, 2):
                                groups.append((range(g0, min(g0 + 2, nja)), 512))
                            for g0 in range(nja, njb, 4):
                                groups.append((range(g0, min(g0 + 4, njb)), 256))

                            deferred = []
                            for t in range(16):
                                hA, hB = _EVENS[t], _ODDS[t]
                                hkA, hkB = hA // 4, hB // 4
                                kbase = (hkA // 2) * 2048
                                rhs_mA = qT[0:64, t * 512 : (t + 1) * 512]
                                rhs_sA = qT[0:64, t * 512 + 256 : (t + 1) * 512]
                                rhs_mB = qT[64:128, t * 512 : (t + 1) * 512]
                                rhs_sB = qT[64:128, t * 512 + 256 : (t + 1) * 512]
                                av_A = PSav.tile([65, 512], F32, tag="av", bufs=3, name="av_A")
                                av_B = PSav.tile([65, 512], F32, tag="av", bufs=3, name="av_B")

                                for h, av in deferred:
                                    emit_gating(h, av)
                                deferred = []

                                # filler qg tiles: 2/pair early, 1/pair late
                                # (late pairs must not underrun or HAM
                                # re-throttles before outproj)
                                filler = []
                                if t < 4:
                                    filler = [6 + t, 22 + t]
                                elif t < 10:
                                    filler = [t + 6]
                                else:
                                    filler = [16 + t]
                                if t == 0 and filler:
                                    # bridge pair-0's sc->exp pipeline
                                    # bootstrap with dense qg work
                                    emit_qg_tile(filler.pop(0))
                                pend = []
                                nflush = 0
                                for jcs, w in groups:
                                    etA, etB = sc_pair_group(
                                        kbase, jcs, w,
                                        rhs_mA if w == 512 else rhs_sA,
                                        rhs_mB if w == 512 else rhs_sB,
                                    )
                                    masks(etA, jcs, w, nja, njb)
                                    masks(etB, jcs, w, nja, njb)
                                    pend.append((jcs, w, etA, etB))
                                    if len(pend) > 1:
                                        jcs0, w0, eA, eB = pend.pop(0)
                                        av_group(av_A, hkA, jcs0, w0, eA, njb)
                                        av_group(av_B, hkB, jcs0, w0, eB, njb)
                                        nflush += 1
                                        if nflush == 1 and filler:
                                            emit_qg_tile(filler.pop(0))
                                        if nflush == 3 and filler:
                                            emit_qg_tile(filler.pop(0))
                                while pend:
                                    jcs0, w0, eA, eB = pend.pop(0)
                                    av_group(av_A, hkA, jcs0, w0, eA, njb)
                                    av_group(av_B, hkB, jcs0, w0, eB, njb)
                                while filler:
                                    emit_qg_tile(filler.pop(0))
                                deferred = [(hA, av_A), (hB, av_B)]
                            for h, av in deferred:
                                emit_gating(h, av)

                    if DEBUG_DUMPS:
                        nc.sync.dma_start(dbg_qT[:], qT[:])
                        nc.sync.dma_start(dbg_sigT[:], sigT[:])
                        nc.sync.dma_start(dbg_kT[:], kT[:])
                        nc.sync.dma_start(dbg_kaug[:], kaug[:])
                        nc.sync.dma_start(dbg_gatedT[:], gatedT[:])

                    # ---- out projection ----
                    with tc.tile_pool(name="psO", bufs=1, space="PSUM") as psO:
                        for oc in range(4):
                            if oc + 1 < 4:
                                load_wo(oc + 1)
                            for mi in range(4):
                                op_ps = psO.tile(
                                    [128, 512], F32, tag="op", bufs=3, name="op_ps"
                                )
                                for fc in range(16):
                                    nc.tensor.matmul(
                                        op_ps[:],
                                        gatedT[:, fc * 512 + mi * 128 : fc * 512 + (mi + 1) * 128],
                                        wo_ts[oc][fc][:],
                                        start=(fc == 0),
                                        stop=(fc == 15),
                                    )
                                o_sb = pO.tile([128, 512], F32, tag="ob", bufs=3, name="o_sb")
                                nc.scalar.copy(o_sb[:], op_ps[:])
                                nc.sync.dma_start(
                                    out[mi * 128 : (mi + 1) * 128, oc * 512 : (oc + 1) * 512],
                                    o_sb[:],
                                )
                            del wo_ts[oc]
    nc.compile()
    return nc


def _get_nc():
    global _NC_CACHE
    if _NC_CACHE is None:
        _NC_CACHE = _build_nc()
    return _NC_CACHE


def _prep_inputs(hidden_states, cos, sin, wq, wk, wo):
    """Build the 8 per-core input maps (all host-side slicing/permutation)."""
    inv = _INV
    dmap = np.concatenate([inv, inv])  # d index for partition p (p%64)
    sign = np.where((np.arange(128) % 64) % 2 == 0, -1.0, 1.0).astype(np.float32)

    wq_q = wq[:, :2048][:, _QCOL_ORDER]
    wq_g = wq[:, 2048:][:, _QCOL_ORDER]
    wqp_flat = np.concatenate([wq_q, wq_g], axis=1)  # [HID, 4096]
    wqp = np.ascontiguousarray(
        wqp_flat.reshape(16, 128, 32, 128).transpose(2, 1, 0, 3).reshape(32 * 128, 16 * 128)
    ).astype(ml_dtypes.bfloat16)
    wkp = np.ascontiguousarray(
        wk.reshape(HID, NKV, D)[:, :, inv].reshape(HID, 512)
    ).astype(ml_dtypes.bfloat16)
    wop = np.ascontiguousarray(wo[_QCOL_ORDER, :]).astype(ml_dtypes.bfloat16)

    in_maps = []
    for c in range(8):
        b, cc = c // 4, c % 4
        r0a, r0b = cc * 256, (7 - cc) * 256
        qrows = np.concatenate([np.arange(r0a, r0a + 256), np.arange(r0b, r0b + 256)])
        xT = hidden_states[b].T  # [HID, S]
        cq = cos[qrows][:, dmap].T  # [128, 512]
        sq = (sin[qrows][:, dmap].T * sign[:, None]).astype(np.float32)
        ck = cos[:, dmap].T  # [128, 2048] all key positions
        sk = (sin[:, dmap].T * sign[:, None]).astype(np.float32)
        in_maps.append(
            {
                "xTq": np.ascontiguousarray(
                    xT[:, qrows].reshape(16, 128, 512).transpose(1, 0, 2).reshape(128, 16 * 512)
                ).astype(ml_dtypes.bfloat16),
                "xTk": np.ascontiguousarray(xT).astype(ml_dtypes.bfloat16),
                "wqp": wqp,
                "wkp": wkp,
                "wop": wop,
                "cosq": np.ascontiguousarray(cq),
                "sinq": np.ascontiguousarray(sq),
                "cosk": np.ascontiguousarray(ck).astype(ml_dtypes.bfloat16),
                "sink": np.ascontiguousarray(sk).astype(ml_dtypes.bfloat16),
            }
        )
    return in_maps


def kernel(hidden_states, cos, sin, attention_mask, wq, wk, wv, wo, **_unused):
    hidden_states = np.asarray(hidden_states, dtype=np.float32)
    cos = np.asarray(cos, dtype=np.float32)
    sin = np.asarray(sin, dtype=np.float32)
    wq = np.asarray(wq, dtype=np.float32)
    wk = np.asarray(wk, dtype=np.float32)
    wo = np.asarray(wo, dtype=np.float32)

    nc = _get_nc()
    in_maps = _prep_inputs(hidden_states, cos, sin, wq, wk, wo)
    res = run_bass_kernel_spmd(nc, in_maps, core_ids=list(range(8)))

    y = np.empty((B, S, HID), dtype=np.float32)
    for c in range(8):
        b, cc = c // 4, c % 4
        r0a, r0b = cc * 256, (7 - cc) * 256
        o = res.results[c]["out"]
        y[b, r0a : r0a + 256] = o[0:256]
        y[b, r0b : r0b + 256] = o[256:512]
    return y



# revision 5
# speedup vs baseline: 1.0083x; 1.0083x over previous
"""Trainium2 Bass kernel for nn_Attention_60739427500161.

Strategy (8 NeuronCores, one chip, no collectives):
- Sequence-sharded (context parallel): core c handles batch b=c//4 and two
  zigzag 256-row query strips (ci*256 and (7-ci)*256, ci=c%4) so causal work
  is balanced. Each core computes q/gate for its 512 rows, the full-batch k
  projection locally, runs attention + gating + out_proj for its rows, and
  writes its 512 output rows. The host scatters them back.
- All matmuls run in bf16. Scores are computed transposed (scoresT[j,i]) so
  softmax needs no PE transposes; denominator l via a ones-column in the av
  matmul; gating computes av / (l * (1 + e^{-g})).
- RoPE rotate_half is a host-side feature permutation so the partner lives
  one partition away and a DVE stream_shuffle([o^1]) produces the rotated
  operand.
- Phase layout: [warmup: 14 qg tiles while wk/xk DMA streams on the scalar
  queue] -> [kproj with transposes interleaved at tf boundaries, 2 qg tiles
  in the transpose tail] -> [16 attention pair-steps, pairs 0-7 carrying 2
  qg-filler tiles each] -> [outproj, wo prefetched during attention].
- Trace-driven scheduling facts baked in (see git/session notes):
  * Issue-to-issue MM gaps run at stream rate (216ns N=512, 109ns N=256,
    30ns transposes) — PE pipelining is fine; stalls are semaphore waits.
  * Causal masking via gpsimd affine_select (~1us each, strict FIFO queue)
    made diagonal av matmuls wait 0.8-1.4us per pair. Replaced with DVE
    tensor_tensor multiplies against two host-provided constant 0/1
    triangle masks (maskd[:, 0:256] = q>=k, [:, 256:512] = q>=k+128).
  * tc.Switch dispatch costs ~4us of PE-queue dead time (two serialized
    DRAM jump-table TENSOR_LOADs). tc.switch_hint at t=0 pre-stages the
    offset + instruction prefetch during the DMA lead-in shadow.
  * Warmup was DMA-bound with head-of-line stalls: wq tiles behind the xq
    bulk on the sync queue. Now: wq0 + tables first, xq in fine chunks,
    wq16 mid-stream, then eager wq pairs (pool-paced, bufs=4).
  * The 11.9us PE gap before kproj (xk tail wait) is filled by 2 extra
    warmup tiles; the transpose tail (HAM sees transpose-mode as idle)
    carries qg tiles 7/23 so the PE clock stays at 8/8 into attention.
  * wo loads used to trickle at 52GB/s during attention and stall outproj
    oc2/3 matmuls 0.5-1.7us each; now 2 sets prefetch at attention start,
    set 2 at pair 10, set 3 during outproj oc0 (scalar queue is idle then).
  * gpsimd ucode library (partition_broadcast) preloaded at t=0: first use
    otherwise costs a ~7us LIBRARY_RELOAD stall mid-pairs.
  * PSUM budget is exactly 8 banks: sc 2x[128,1024] + av 3x[65,512] + qg 1.
- wqp/xTq are host-pre-laid-out so every wq/xq DMA is a dense
  4-16KB-per-partition transfer.
- Chip-level P0 power-state variance is real: identical binaries measure
  +/-9% run to run (2.0 vs 2.4GHz PE clock under sustained load).
"""

import sys

for _p in ("/root/.axon_site/_ro/trn_rl_repo", "/opt/trn_rl_repo"):
    if _p not in sys.path:
        sys.path.append(_p)

import ml_dtypes
import numpy as np

import concourse.bass as bass
import concourse.mybir as mybir
import concourse.tile as tile
from concourse import bacc
from concourse.bass_utils import run_bass_kernel_spmd
from concourse.masks import make_identity

F32 = mybir.dt.float32
BF16 = mybir.dt.bfloat16
AF = mybir.ActivationFunctionType
ALU = mybir.AluOpType
ET = mybir.EngineType

B, S, HID = 2, 2048, 2048
NH, NKV, D = 32, 8, 64

# pi permutation: interleave (d, d+32) pairs so rotate_half partner is the
# adjacent partition. pos(d) = 2d (d<32) else 2(d-32)+1.
_POS = np.array([2 * d if d < 32 else 2 * (d - 32) + 1 for d in range(D)])
_INV = np.argsort(_POS)
_SHUF_MASK = [o ^ 1 for o in range(32)]

# q-head placement: head h must share its SBUF row base (0 or 64) with its
# kv head hk=h//4. Tile t pairs one even-hk head (rows 0-63) with one odd-hk
# head (rows 64-127); both heads of tile t form attention pair t.
_EVENS = [h for h in range(NH) if (h // 4) % 2 == 0]
_ODDS = [h for h in range(NH) if (h // 4) % 2 == 1]
_QTILE = [0] * NH
_QROW = [0] * NH
for _i, _h in enumerate(_EVENS):
    _QTILE[_h], _QROW[_h] = _i, 0
for _i, _h in enumerate(_ODDS):
    _QTILE[_h], _QROW[_h] = _i, 64
_QCOL_ORDER = np.concatenate(
    [np.concatenate([_EVENS[t] * D + _INV, _ODDS[t] * D + _INV]) for t in range(16)]
)

_NC_CACHE = None
N_WARMUP = 7  # qg tile-pairs (q+gate) emitted before kproj


def _build_nc():
    nc = bacc.Bacc(None, target_bir_lowering=False, enable_partition_id=True)

    xTq = nc.dram_tensor("xTq", [128, 16 * 512], BF16, kind="ExternalInput")
    xTk = nc.dram_tensor("xTk", [HID, S], BF16, kind="ExternalInput")
    wqp = nc.dram_tensor("wqp", [32 * 128, 16 * 128], BF16, kind="ExternalInput")
    wkp = nc.dram_tensor("wkp", [HID, 512], BF16, kind="ExternalInput")
    wop = nc.dram_tensor("wop", [HID, HID], BF16, kind="ExternalInput")
    cosq = nc.dram_tensor("cosq", [128, 512], F32, kind="ExternalInput")
    sinq = nc.dram_tensor("sinq", [128, 512], F32, kind="ExternalInput")
    cosk = nc.dram_tensor("cosk", [128, 2048], BF16, kind="ExternalInput")
    sink = nc.dram_tensor("sink", [128, 2048], BF16, kind="ExternalInput")
    maskd = nc.dram_tensor("maskd", [128, 512], BF16, kind="ExternalInput")
    out = nc.dram_tensor("out", [512, HID], F32, kind="ExternalOutput")

    with tile.TileContext(nc) as tc:
        ci = nc.partition_id() % 4
        # Pre-stage the attention Switch dispatch (offset regs + branch
        # prefetch) under the DMA lead-in shadow; the dispatch itself then
        # avoids ~4us of serialized DRAM jump-table loads on the PE queue.
        sw_hint = tc.switch_hint(
            index={e: ci for e in (ET.PE, ET.DVE, ET.Activation, ET.Pool, ET.SP)},
            n=4,
        )

        with tc.tile_pool(name="persist", bufs=1) as pers:
            qT = pers.tile([128, 16 * 512], BF16, tag="qT")
            sigT = pers.tile([128, 16 * 512], BF16, tag="sigT")
            kT = pers.tile([128, 4 * 2048], BF16, tag="kT")
            kaug = pers.tile([128, NKV * 16 * 65], BF16, tag="kaug")
            kaug4 = kaug[:].rearrange("p (h j d) -> p h j d", h=NKV, j=16)
            mask_sb = pers.tile([128, 512], BF16, tag="maskd")

            ident = pers.tile([128, 64], BF16, tag="ident")
            make_identity(nc, ident[0:64, :])
            nc.sync.dma_start(ident[64:128, :], ident[0:64, :])

            # Load the gpsimd partition_broadcast ucode library NOW, under
            # the DMA lead-in shadow: first use otherwise costs a ~7us
            # LIBRARY_RELOAD stall mid-pairs.
            gpw = pers.tile([2, 64], F32, tag="gpw")
            nc.vector.memset(gpw[:], 0.0)
            nc.gpsimd.partition_broadcast(gpw[:], gpw[0:1, :])

            with (
                tc.tile_pool(name="pXq", bufs=1) as pXq,
                tc.tile_pool(name="pWq", bufs=2) as pWq,
                tc.tile_pool(name="pRt", bufs=2) as pRt,
                tc.tile_pool(name="PSqg", bufs=1, space="PSUM") as PSqg,
            ):
                # ---- warmup-stream DMAs (sync queue) ----
                # Order: first tile's wq + RoPE tables, xq in fine chunks
                # (every qg tile contracts over ALL of xq, so xq lands as
                # early as possible), wq16 mid-stream, then eager wq pairs.
                # pWq bufs=4 paces the eager loads to consumption order.
                xq = pXq.tile([128, 16 * 512], BF16, tag="xq")

                def load_wq(t):
                    # host pre-layout (t, p, kc, m): one dense 4KB/partition
                    # transfer (the old (kc p) m gather read 256B segments)
                    wq_t = pWq.tile([128, 16 * 128], BF16, tag="wq", bufs=4, name="wq_t")
                    nc.sync.dma_start(wq_t[:], wqp[t * 128 : (t + 1) * 128, :])
                    return wq_t

                wq_pre = {0: load_wq(0)}
                cosq_sb = pXq.tile([128, 512], F32, tag="cosq")
                sinq_sb = pXq.tile([128, 512], F32, tag="sinq")
                nc.sync.dma_start(cosq_sb[:], cosq[:])
                nc.sync.dma_start(sinq_sb[:], sinq[:])
                for xc in range(0, 6, 2):
                    nc.sync.dma_start(
                        xq[:, xc * 512 : (xc + 2) * 512], xTq[:, xc * 512 : (xc + 2) * 512]
                    )
                wq_pre[16] = load_wq(16)
                for xc in range(6, 16, 2):
                    nc.sync.dma_start(
                        xq[:, xc * 512 : (xc + 2) * 512], xTq[:, xc * 512 : (xc + 2) * 512]
                    )
                for _t in range(1, N_WARMUP):
                    wq_pre[_t] = load_wq(_t)
                    wq_pre[16 + _t] = load_wq(16 + _t)

                def emit_qg_tile(t, ps_pool=None, ps_bufs=1):
                    """qg projection m-tile t (q-tile if t<16 else gate)."""
                    wq_t = wq_pre.pop(t) if t in wq_pre else load_wq(t)
                    qg_ps = (ps_pool or PSqg).tile(
                        [128, 512], F32, tag="qg", bufs=ps_bufs, name="qg_ps"
                    )
                    for kc in range(16):
                        nc.tensor.matmul(
                            qg_ps[:],
                            wq_t[:, kc * 128 : (kc + 1) * 128],
                            xq[:, kc * 512 : (kc + 1) * 512],
                            start=(kc == 0),
                            stop=(kc == 15),
                        )
                    if t < 16:
                        shf = pRt.tile([128, 512], F32, tag="shf", name="shf")
                        nc.vector.stream_shuffle(shf[:], qg_ps[:], _SHUF_MASK)
                        t1 = pRt.tile([128, 512], F32, tag="t1", name="t1")
                        nc.vector.tensor_tensor(t1[:], qg_ps[:], cosq_sb[:], ALU.mult)
                        t2 = pRt.tile([128, 512], F32, tag="t2", name="t2")
                        nc.vector.tensor_tensor(t2[:], shf[:], sinq_sb[:], ALU.mult)
                        nc.vector.tensor_tensor(
                            qT[:, t * 512 : (t + 1) * 512], t1[:], t2[:], ALU.add
                        )
                    else:
                        # e^{-g}; 1/(1+e^{-g}) is folded into the gating recip
                        nc.scalar.activation(
                            sigT[:, (t - 16) * 512 : (t - 15) * 512],
                            qg_ps[:],
                            AF.Exp,
                            scale=-1.0,
                        )

                # ---- phase A DMAs on the SCALAR queue so the warmup-stream
                # wq tile DMAs (pool-gated) can't head-of-line block them ----
                with (
                    tc.tile_pool(name="pAtab", bufs=1) as pAtab,
                    tc.tile_pool(name="pA", bufs=1) as pA,
                ):
                    wk_all = pA.tile([128, 16 * 512], BF16, tag="wk")
                    xk_all = pA.tile([128, 16 * 2048], BF16, tag="xk")
                    for kh in range(16):
                        nc.scalar.dma_start(
                            wk_all[:, kh * 512 : (kh + 1) * 512],
                            wkp[kh * 128 : (kh + 1) * 128, :],
                        )
                        nc.scalar.dma_start(
                            xk_all[:, kh * 2048 : (kh + 1) * 2048],
                            xTk[kh * 128 : (kh + 1) * 128, :],
                        )
                    cosk_sb = pAtab.tile([128, 2048], BF16, tag="cosk")
                    sink_sb = pAtab.tile([128, 2048], BF16, tag="sink")
                    nc.scalar.dma_start(cosk_sb[:], cosk[:])
                    nc.scalar.dma_start(sink_sb[:], sink[:])
                    nc.scalar.dma_start(mask_sb[:], maskd[:])

                    # ---- warmup: qg tiles run while wk/xk stream in. ----
                    with nc.named_scope("warmup"):
                        with tc.tile_pool(name="PSwarm", bufs=1, space="PSUM") as PSwarm:
                            for w in range(N_WARMUP):
                                emit_qg_tile(w, PSwarm, 2)
                                emit_qg_tile(16 + w, PSwarm, 2)

                    # ---- kproj (dense: xk fully resident by now) ----
                    with nc.named_scope("kproj"), (
                        tc.tile_pool(name="psA", bufs=1, space="PSUM")
                    ) as psA:
                        def emit_transposes(hk):
                            """kaug chunks for kv head hk from the RoPE'd kT.
                            Interleaved at tf boundaries so dense kproj MMs
                            surround the transpose-mode stretches (HAM does
                            not count transpose-mode as PE-busy)."""
                            hkr = (hk % 2) * 64
                            base = (hk // 2) * 2048
                            for jj in range(4):
                                tr = psA.tile([128, 256], BF16, tag="tr", bufs=2, name="tr")
                                for u in range(4):
                                    jc = jj * 4 + u
                                    nc.tensor.transpose(
                                        tr[:, u * 64 : (u + 1) * 64],
                                        kT[hkr : hkr + 64, base + jc * 128 : base + (jc + 1) * 128],
                                        ident[hkr : hkr + 64, :],
                                    )
                                nc.vector.tensor_copy(
                                    kaug4[:, hk, jj * 4 : (jj + 1) * 4, 0:64],
                                    tr[:].rearrange("p (u d) -> p u d", u=4),
                                )

                        for tf in range(4):
                            for kb in range(4):
                                kp_ps = psA.tile(
                                    [128, 512], F32, tag="kp", bufs=2, name="kp_ps"
                                )
                                for kc in range(16):
                                    nc.tensor.matmul(
                                        kp_ps[:],
                                        wk_all[:, kc * 512 + tf * 128 : kc * 512 + (tf + 1) * 128],
                                        xk_all[:, kc * 2048 + kb * 512 : kc * 2048 + (kb + 1) * 512],
                                        start=(kc == 0),
                                        stop=(kc == 15),
                                    )
                                shf = pRt.tile([128, 512], F32, tag="shf")
                                nc.vector.stream_shuffle(shf[:], kp_ps[:], _SHUF_MASK)
                                t1 = pRt.tile([128, 512], F32, tag="t1")
                                nc.vector.tensor_tensor(
                                    t1[:], kp_ps[:], cosk_sb[:, kb * 512 : (kb + 1) * 512], ALU.mult
                                )
                                t2 = pRt.tile([128, 512], F32, tag="t2")
                                nc.vector.tensor_tensor(
                                    t2[:], shf[:], sink_sb[:, kb * 512 : (kb + 1) * 512], ALU.mult
                                )
                                nc.vector.tensor_tensor(
                                    kT[:, tf * 2048 + kb * 512 : tf * 2048 + (kb + 1) * 512],
                                    t1[:],
                                    t2[:],
                                    ALU.add,
                                )
                            if tf >= 1:
                                emit_transposes(2 * (tf - 1))
                                emit_transposes(2 * (tf - 1) + 1)
                        # dense qg work between the tail transpose stretches
                        # keeps HAM at 8/8 into attention (transpose-mode
                        # reads as PE-idle to the clock governor)
                        emit_qg_tile(N_WARMUP)
                        emit_transposes(6)
                        emit_qg_tile(16 + N_WARMUP)
                        emit_transposes(7)
                nc.vector.memset(kaug4[:, :, :, 64:65], 1.0)

                # ---- attention pair-steps ----
                with (
                    tc.tile_pool(name="pG", bufs=1) as pG,
                    tc.tile_pool(name="pO", bufs=3) as pO,
                ):
                    gatedT = pG.tile([128, 16 * 512], BF16, tag="gatedT")

                    wo_ts = {}
                    def load_wo(oc):
                        lst = []
                        for fc in range(16):
                            wo_t = pO.tile([128, 512], BF16, tag="wo", bufs=48, name="wo_t")
                            nc.scalar.dma_start(
                                wo_t[:],
                                wop[fc * 128 : (fc + 1) * 128, oc * 512 : (oc + 1) * 512],
                            )
                            lst.append(wo_t)
                        wo_ts[oc] = lst
                    # prefetch outproj wo during attention (scalar DMA queue
                    # is idle after kproj inputs land)
                    load_wo(0)
                    load_wo(1)

                    with nc.named_scope("attn"), (
                        tc.tile_pool(name="pET", bufs=5)
                    ) as pET, (
                        tc.tile_pool(name="pSm", bufs=1)
                    ) as pSm, (
                        tc.tile_pool(name="PSsc", bufs=1, space="PSUM")
                    ) as PSsc, (
                        tc.tile_pool(name="PSav", bufs=1, space="PSUM")
                    ) as PSav:
                        def sc_pair_group(kbase, jcs, w, rhs_A, rhs_B):
                          """Score matmuls + exp for one chunk group of BOTH
                          heads, MMs interleaved A/B so consecutive PE instrs
                          target opposite row-tiles (0,0)/(64,0) and their
                          LDWEIGHTS can pull ahead past the other tile's
                          in-flight matmul."""
                          scA = PSsc.tile([128, 1024], F32, tag="sc", bufs=2, name="scA")
                          scB = PSsc.tile([128, 1024], F32, tag="sc", bufs=2, name="scB")
                          for u, jc in enumerate(jcs):
                              nc.tensor.matmul(
                                  scA[:, u * w : u * w + w],
                                  kT[0:64, kbase + jc * 128 : kbase + (jc + 1) * 128],
                                  rhs_A,
                                  start=True,
                                  stop=True,
                              )
                              nc.tensor.matmul(
                                  scB[:, u * w : u * w + w],
                                  kT[64:128, kbase + jc * 128 : kbase + (jc + 1) * 128],
                                  rhs_B,
                                  start=True,
                                  stop=True,
                              )
                          n_tot = len(jcs) * w
                          etA = pET.tile([128, 1024], BF16, tag="et", name="etA")
                          nc.scalar.activation(etA[:, 0:n_tot], scA[:, 0:n_tot], AF.Exp)
                          etB = pET.tile([128, 1024], BF16, tag="et", name="etB")
                          nc.scalar.activation(etB[:, 0:n_tot], scB[:, 0:n_tot], AF.Exp)
                          return etA, etB

                        def masks(et, jcs, w, nja, njb):
                            # causal triangle masks: DVE multiply against the
                            # host-built constants (mask_sb[:,0:256] = q>=k,
                            # [:,256:512] = q>=k+128); keeps the strict-FIFO
                            # gpsimd queue out of the av critical path
                            for u, jc in enumerate(jcs):
                                if jc >= nja - 2 and jc < nja and w == 512:
                                    mi = 0 if jc == nja - 2 else 1
                                    nc.vector.tensor_tensor(
                                        et[:, u * w : u * w + 256],
                                        et[:, u * w : u * w + 256],
                                        mask_sb[:, mi * 256 : (mi + 1) * 256],
                                        ALU.mult,
                                    )
                                if jc >= njb - 2:
                                    off = u * w + (256 if w == 512 else 0)
                                    mi = 0 if jc == njb - 2 else 1
                                    nc.vector.tensor_tensor(
                                        et[:, off : off + 256],
                                        et[:, off : off + 256],
                                        mask_sb[:, mi * 256 : (mi + 1) * 256],
                                        ALU.mult,
                                    )

                        def av_group(av_ps, hk, jcs, w, et, njb):
                            for u, jc in enumerate(jcs):
                                nc.tensor.matmul(
                                    av_ps[0:65, (0 if w == 512 else 256) : 512],
                                    kaug[:, (hk * 16 + jc) * 65 : (hk * 16 + jc) * 65 + 65],
                                    et[:, u * w : u * w + w],
                                    start=(jc == 0),
                                    stop=(jc == njb - 1),
                                    skip_group_check=True,
                                )

                        def emit_gating(h, av_ps):
                            # gated = av / (l * (1 + e^{-g}))
                            tq, hr = _QTILE[h], _QROW[h]
                            lrow = pSm.tile([1, 512], F32, tag="lrow", name="lrow")
                            nc.vector.tensor_copy(lrow[:], av_ps[64:65, :])
                            lb = pSm.tile([64, 512], F32, tag="lb", name="lb")
                            nc.gpsimd.partition_broadcast(lb[:], lrow[:])
                            eg = sigT[hr : hr + 64, tq * 512 : (tq + 1) * 512]
                            if hr:
                                egc = pSm.tile([64, 512], BF16, tag="egc", name="egc")
                                nc.vector.tensor_copy(egc[:], eg)
                                eg = egc[:]
                            den = pSm.tile([64, 512], F32, tag="den", name="den")
                            nc.vector.scalar_tensor_tensor(
                                den[:], eg, 1.0, lb[:], ALU.add, ALU.mult
                            )
                            rden = pSm.tile([64, 512], F32, tag="rden", name="rden")
                            nc.vector.reciprocal_approx_fast(rden[:], den[:])
                            nc.vector.tensor_tensor(
                                gatedT[hr : hr + 64, tq * 512 : (tq + 1) * 512],
                                av_ps[0:64, :],
                                rden[:],
                                ALU.mult,
                            )

                        for arm in tc.Switch(ci, 4, hint=sw_hint):
                            nja, njb = 2 * arm + 2, 16 - 2 * arm
                            groups = []
                            for g0 in range(0, nja, 2):
                                groups.append((range(g0, min(g0 + 2, nja)), 512))
                            for g0 in range(nja, njb, 4):
                                groups.append((range(g0, min(g0 + 4, njb)), 256))

                            deferred = []
                            for t in range(16):
                                hA, hB = _EVENS[t], _ODDS[t]
                                hkA, hkB = hA // 4, hB // 4
                                kbase = (hkA // 2) * 2048
                                rhs_mA = qT[0:64, t * 512 : (t + 1) * 512]
                                rhs_sA = qT[0:64, t * 512 + 256 : (t + 1) * 512]
                                rhs_mB = qT[64:128, t * 512 : (t + 1) * 512]
                                rhs_sB = qT[64:128, t * 512 + 256 : (t + 1) * 512]
                                av_A = PSav.tile([65, 512], F32, tag="av", bufs=3, name="av_A")
                                av_B = PSav.tile([65, 512], F32, tag="av", bufs=3, name="av_B")

                                for h, av in deferred:
                                    emit_gating(h, av)
                                deferred = []

                                # filler qg tiles: pairs 0-7 carry the q tile
                                # for pair t+8 and the gate tile for pair
                                # t+8's gating (runs at pair t+9)
                                filler = []
                                if t < 8:
                                    filler = [(N_WARMUP + 1) + t, (16 + N_WARMUP + 1) + t]
                                if t == 0 and filler:
                                    # bridge pair-0's sc->exp pipeline
                                    # bootstrap with dense qg work
                                    emit_qg_tile(filler.pop(0))
                                pend = []
                                nflush = 0
                                for jcs, w in groups:
                                    etA, etB = sc_pair_group(
                                        kbase, jcs, w,
                                        rhs_mA if w == 512 else rhs_sA,
                                        rhs_mB if w == 512 else rhs_sB,
                                    )
                                    masks(etA, jcs, w, nja, njb)
                                    masks(etB, jcs, w, nja, njb)
                                    pend.append((jcs, w, etA, etB))
                                    if len(pend) > 1:
                                        jcs0, w0, eA, eB = pend.pop(0)
                                        av_group(av_A, hkA, jcs0, w0, eA, njb)
                                        av_group(av_B, hkB, jcs0, w0, eB, njb)
                                        nflush += 1
                                        if nflush == 1 and filler:
                                            emit_qg_tile(filler.pop(0))
                                        if nflush == 3 and filler:
                                            emit_qg_tile(filler.pop(0))
                                while len(pend) > 1:
                                    jcs0, w0, eA, eB = pend.pop(0)
                                    av_group(av_A, hkA, jcs0, w0, eA, njb)
                                    av_group(av_B, hkB, jcs0, w0, eB, njb)
                                jcs0, w0, eA, eB = pend.pop(0)
                                av_group(av_A, hkA, jcs0, w0, eA, njb)
                                if t == 15:
                                    # last pair: gate A while B's final av
                                    # group runs so outproj isn't serialized
                                    # behind both gating chains
                                    emit_gating(hA, av_A)
                                    av_group(av_B, hkB, jcs0, w0, eB, njb)
                                    emit_gating(hB, av_B)
                                else:
                                    av_group(av_B, hkB, jcs0, w0, eB, njb)
                                    while filler:
                                        emit_qg_tile(filler.pop(0))
                                    deferred = [(hA, av_A), (hB, av_B)]
                            for h, av in deferred:
                                emit_gating(h, av)

                    # ---- out projection ----
                    with nc.named_scope("outproj"), (
                        tc.tile_pool(name="psO", bufs=1, space="PSUM")
                    ) as psO:
                        for oc in range(4):
                            if oc == 0:
                                # sets 2+3 stream during oc0/oc1 compute on
                                # the otherwise-idle scalar queue
                                load_wo(2)
                                load_wo(3)
                            for mi in range(4):
                                op_ps = psO.tile(
                                    [128, 512], F32, tag="op", bufs=3, name="op_ps"
                                )
                                for fc in range(16):
                                    nc.tensor.matmul(
                                        op_ps[:],
                                        gatedT[:, fc * 512 + mi * 128 : fc * 512 + (mi + 1) * 128],
                                        wo_ts[oc][fc][:],
                                        start=(fc == 0),
                                        stop=(fc == 15),
                                    )
                                o_sb = pO.tile([128, 512], F32, tag="ob", bufs=3, name="o_sb")
                                if oc == 3 and mi == 3:
                                    # split the tail copy+DMA so the final
                                    # write starts before the full copy ends
                                    for hh in range(2):
                                        nc.scalar.copy(
                                            o_sb[:, hh * 256 : (hh + 1) * 256],
                                            op_ps[:, hh * 256 : (hh + 1) * 256],
                                        )
                                        nc.sync.dma_start(
                                            out[
                                                mi * 128 : (mi + 1) * 128,
                                                oc * 512 + hh * 256 : oc * 512 + (hh + 1) * 256,
                                            ],
                                            o_sb[:, hh * 256 : (hh + 1) * 256],
                                        )
                                else:
                                    nc.scalar.copy(o_sb[:], op_ps[:])
                                    nc.sync.dma_start(
                                        out[mi * 128 : (mi + 1) * 128, oc * 512 : (oc + 1) * 512],
                                        o_sb[:],
                                    )
                            del wo_ts[oc]
    nc.compile()
    return nc


def _get_nc():
    global _NC_CACHE
    if _NC_CACHE is None:
        _NC_CACHE = _build_nc()
    return _NC_CACHE


def _prep_inputs(hidden_states, cos, sin, wq, wk, wo):
    """Build the 8 per-core input maps (all host-side slicing/permutation)."""
    inv = _INV
    dmap = np.concatenate([inv, inv])  # d index for partition p (p%64)
    sign = np.where((np.arange(128) % 64) % 2 == 0, -1.0, 1.0).astype(np.float32)

    wq_q = wq[:, :2048][:, _QCOL_ORDER]
    wq_g = wq[:, 2048:][:, _QCOL_ORDER]
    wqp_flat = np.concatenate([wq_q, wq_g], axis=1)  # [HID, 4096]
    wqp = np.ascontiguousarray(
        wqp_flat.reshape(16, 128, 32, 128).transpose(2, 1, 0, 3).reshape(32 * 128, 16 * 128)
    ).astype(ml_dtypes.bfloat16)
    wkp = np.ascontiguousarray(
        wk.reshape(HID, NKV, D)[:, :, inv].reshape(HID, 512)
    ).astype(ml_dtypes.bfloat16)
    wop = np.ascontiguousarray(wo[_QCOL_ORDER, :]).astype(ml_dtypes.bfloat16)

    # causal triangle masks for the diagonal 128x256 blocks:
    # maskd[:, 0:256][k, q] = 1 iff q >= k ; [:, 256:512][k, q] = 1 iff q >= k+128
    karr = np.arange(128)[:, None]
    qarr = np.arange(256)[None, :]
    maskd = np.concatenate(
        [(qarr >= karr), (qarr >= karr + 128)], axis=1
    ).astype(ml_dtypes.bfloat16)
    maskd = np.ascontiguousarray(maskd)

    in_maps = []
    for c in range(8):
        b, cc = c // 4, c % 4
        r0a, r0b = cc * 256, (7 - cc) * 256
        qrows = np.concatenate([np.arange(r0a, r0a + 256), np.arange(r0b, r0b + 256)])
        xT = hidden_states[b].T  # [HID, S]
        cq = cos[qrows][:, dmap].T  # [128, 512]
        sq = (sin[qrows][:, dmap].T * sign[:, None]).astype(np.float32)
        ck = cos[:, dmap].T  # [128, 2048] all key positions
        sk = (sin[:, dmap].T * sign[:, None]).astype(np.float32)
        in_maps.append(
            {
                "xTq": np.ascontiguousarray(
                    xT[:, qrows].reshape(16, 128, 512).transpose(1, 0, 2).reshape(128, 16 * 512)
                ).astype(ml_dtypes.bfloat16),
                "xTk": np.ascontiguousarray(xT).astype(ml_dtypes.bfloat16),
                "wqp": wqp,
                "wkp": wkp,
                "wop": wop,
                "cosq": np.ascontiguousarray(cq),
                "sinq": np.ascontiguousarray(sq),
                "cosk": np.ascontiguousarray(ck).astype(ml_dtypes.bfloat16),
                "sink": np.ascontiguousarray(sk).astype(ml_dtypes.bfloat16),
                "maskd": maskd,
            }
        )
    return in_maps


def kernel(hidden_states, cos, sin, attention_mask, wq, wk, wv, wo, **_unused):
    hidden_states = np.asarray(hidden_states, dtype=np.float32)
    cos = np.asarray(cos, dtype=np.float32)
    sin = np.asarray(sin, dtype=np.float32)
    wq = np.asarray(wq, dtype=np.float32)
    wk = np.asarray(wk, dtype=np.float32)
    wo = np.asarray(wo, dtype=np.float32)

    nc = _get_nc()
    in_maps = _prep_inputs(hidden_states, cos, sin, wq, wk, wo)
    res = run_bass_kernel_spmd(nc, in_maps, core_ids=list(range(8)))

    y = np.empty((B, S, HID), dtype=np.float32)
    for c in range(8):
        b, cc = c // 4, c % 4
        r0a, r0b = cc * 256, (7 - cc) * 256
        o = res.results[c]["out"]
        y[b, r0a : r0a + 256] = o[0:256]
        y[b, r0b : r0b + 256] = o[256:512]
    return y


# revision 9
# speedup vs baseline: 1.0502x; 1.0416x over previous
"""Trainium2 Bass kernel for nn_Attention_60739427500161.

Strategy (8 NeuronCores, one chip, no collectives):
- Sequence-sharded (context parallel): core c handles batch b=c//4 and two
  zigzag 256-row query strips (ci*256 and (7-ci)*256, ci=c%4) so causal work
  is balanced. Each core computes q/gate for its 512 rows, the full-batch k
  projection locally, runs attention + gating + out_proj for its rows, and
  writes its 512 output rows. The host scatters them back.
- All matmuls run in bf16. Scores are computed transposed (scoresT[j,i]) so
  softmax needs no PE transposes; denominator l via a ones-column in the av
  matmul; gating computes av / (l * (1 + e^{-g})).
- RoPE rotate_half is a host-side feature permutation so the partner lives
  one partition away and a DVE stream_shuffle([o^1]) produces the rotated
  operand.
- Phase layout: [warmup: 14 qg tiles while wk/xk DMA streams on the scalar
  queue] -> [kproj with transposes interleaved at tf boundaries, 2 qg tiles
  in the transpose tail] -> [16 attention pair-steps, pairs 0-7 carrying 2
  qg-filler tiles each] -> [outproj, wo prefetched during attention].
- Trace-driven scheduling facts baked in (see git/session notes):
  * Issue-to-issue MM gaps run at stream rate (216ns N=512, 109ns N=256,
    30ns transposes) — PE pipelining is fine; stalls are semaphore waits.
  * Causal masking via gpsimd affine_select (~1us each, strict FIFO queue)
    made diagonal av matmuls wait 0.8-1.4us per pair. Replaced with DVE
    tensor_tensor multiplies against two host-provided constant 0/1
    triangle masks (maskd[:, 0:256] = q>=k, [:, 256:512] = q>=k+128).
  * tc.Switch dispatch costs ~4us of PE-queue dead time (two serialized
    DRAM jump-table TENSOR_LOADs). tc.switch_hint at t=0 pre-stages the
    offset + instruction prefetch during the DMA lead-in shadow.
  * Warmup was DMA-bound with head-of-line stalls: wq tiles behind the xq
    bulk on the sync queue. Now: wq0 + tables first, xq in fine chunks,
    wq16 mid-stream, then eager wq pairs (pool-paced, bufs=4).
  * The 11.9us PE gap before kproj (xk tail wait) is filled by 2 extra
    warmup tiles; the transpose tail (HAM sees transpose-mode as idle)
    carries qg tiles 7/23 so the PE clock stays at 8/8 into attention.
  * wo loads used to trickle at 52GB/s during attention and stall outproj
    oc2/3 matmuls 0.5-1.7us each; now 2 sets prefetch at attention start,
    set 2 at pair 10, set 3 during outproj oc0 (scalar queue is idle then).
  * gpsimd ucode library (partition_broadcast) preloaded at t=0: first use
    otherwise costs a ~7us LIBRARY_RELOAD stall mid-pairs.
  * PSUM budget is exactly 8 banks: sc 2x[128,1024] + av 3x[65,512] + qg 1.
- wqp/xTq are host-pre-laid-out so every wq/xq DMA is a dense
  4-16KB-per-partition transfer.
- Chip-level P0 power-state variance is real: identical binaries measure
  +/-9% run to run (2.0 vs 2.4GHz PE clock under sustained load).
"""

import sys

for _p in ("/root/.axon_site/_ro/trn_rl_repo", "/opt/trn_rl_repo"):
    if _p not in sys.path:
        sys.path.append(_p)

import ml_dtypes
import numpy as np

import concourse.bass as bass
import concourse.mybir as mybir
import concourse.tile as tile
from concourse import bacc
from concourse.bass_utils import run_bass_kernel_spmd
from concourse.masks import make_identity

F32 = mybir.dt.float32
BF16 = mybir.dt.bfloat16
AF = mybir.ActivationFunctionType
ALU = mybir.AluOpType
ET = mybir.EngineType

B, S, HID = 2, 2048, 2048
NH, NKV, D = 32, 8, 64

# pi permutation: interleave (d, d+32) pairs so rotate_half partner is the
# adjacent partition. pos(d) = 2d (d<32) else 2(d-32)+1.
_POS = np.array([2 * d if d < 32 else 2 * (d - 32) + 1 for d in range(D)])
_INV = np.argsort(_POS)
_SHUF_MASK = [o ^ 1 for o in range(32)]

# q-head placement: head h must share its SBUF row base (0 or 64) with its
# kv head hk=h//4. Tile t pairs one even-hk head (rows 0-63) with one odd-hk
# head (rows 64-127); both heads of tile t form attention pair t.
_EVENS = [h for h in range(NH) if (h // 4) % 2 == 0]
_ODDS = [h for h in range(NH) if (h // 4) % 2 == 1]
_QTILE = [0] * NH
_QROW = [0] * NH
for _i, _h in enumerate(_EVENS):
    _QTILE[_h], _QROW[_h] = _i, 0
for _i, _h in enumerate(_ODDS):
    _QTILE[_h], _QROW[_h] = _i, 64
_QCOL_ORDER = np.concatenate(
    [np.concatenate([_EVENS[t] * D + _INV, _ODDS[t] * D + _INV]) for t in range(16)]
)

_NC_CACHE = None
N_WARMUP = 7  # qg tile-pairs (q+gate) emitted before kproj


def _build_nc():
    nc = bacc.Bacc(None, target_bir_lowering=False, enable_partition_id=True)

    xTq = nc.dram_tensor("xTq", [128, 16 * 512], BF16, kind="ExternalInput")
    xTk = nc.dram_tensor("xTk", [HID, S], BF16, kind="ExternalInput")
    wqp = nc.dram_tensor("wqp", [32 * 128, 16 * 128], BF16, kind="ExternalInput")
    wkp = nc.dram_tensor("wkp", [HID, 512], BF16, kind="ExternalInput")
    wop = nc.dram_tensor("wop", [HID, HID], BF16, kind="ExternalInput")
    cosq = nc.dram_tensor("cosq", [128, 512], F32, kind="ExternalInput")
    sinq = nc.dram_tensor("sinq", [128, 512], F32, kind="ExternalInput")
    cosk = nc.dram_tensor("cosk", [128, 2048], BF16, kind="ExternalInput")
    sink = nc.dram_tensor("sink", [128, 2048], BF16, kind="ExternalInput")
    maskd = nc.dram_tensor("maskd", [128, 512], BF16, kind="ExternalInput")
    out = nc.dram_tensor("out", [512, HID], F32, kind="ExternalOutput")

    with tile.TileContext(nc) as tc:
        ci = nc.partition_id() % 4
        # Pre-stage the attention Switch dispatch (offset regs + branch
        # prefetch) under the DMA lead-in shadow; the dispatch itself then
        # avoids ~4us of serialized DRAM jump-table loads on the PE queue.
        sw_hint = tc.switch_hint(
            index={e: ci for e in (ET.PE, ET.DVE, ET.Activation, ET.Pool, ET.SP)},
            n=4,
        )

        with tc.tile_pool(name="persist", bufs=1) as pers:
            qT = pers.tile([128, 16 * 512], BF16, tag="qT")
            sigT = pers.tile([128, 16 * 512], BF16, tag="sigT")
            kT = pers.tile([128, 4 * 2048], BF16, tag="kT")
            kaug = pers.tile([128, NKV * 16 * 65], BF16, tag="kaug")
            kaug4 = kaug[:].rearrange("p (h j d) -> p h j d", h=NKV, j=16)
            mask_sb = pers.tile([128, 512], BF16, tag="maskd")

            ident = pers.tile([128, 64], BF16, tag="ident")
            make_identity(nc, ident[0:64, :])
            nc.sync.dma_start(ident[64:128, :], ident[0:64, :])

            # Load the gpsimd partition_broadcast ucode library NOW, under
            # the DMA lead-in shadow: first use otherwise costs a ~7us
            # LIBRARY_RELOAD stall mid-pairs.
            gpw = pers.tile([2, 64], F32, tag="gpw")
            nc.vector.memset(gpw[:], 0.0)
            nc.gpsimd.partition_broadcast(gpw[:], gpw[0:1, :])

            with (
                tc.tile_pool(name="pXq", bufs=1) as pXq,
                tc.tile_pool(name="pWq", bufs=2) as pWq,
                tc.tile_pool(name="pRt", bufs=2) as pRt,
                tc.tile_pool(name="PSqg", bufs=1, space="PSUM") as PSqg,
            ):
                # ---- warmup-stream DMAs (sync queue) ----
                # Order: first tile's wq + RoPE tables, xq in fine chunks
                # (every qg tile contracts over ALL of xq, so xq lands as
                # early as possible), wq16 mid-stream, then eager wq pairs.
                # pWq bufs=4 paces the eager loads to consumption order.
                xq = pXq.tile([128, 16 * 512], BF16, tag="xq")

                def load_wq(t):
                    # host pre-layout (t, p, kc, m): one dense 4KB/partition
                    # transfer (the old (kc p) m gather read 256B segments)
                    wq_t = pWq.tile([128, 16 * 128], BF16, tag="wq", bufs=4, name="wq_t")
                    nc.sync.dma_start(wq_t[:], wqp[t * 128 : (t + 1) * 128, :])
                    return wq_t

                wq_pre = {0: load_wq(0)}
                cosq_sb = pXq.tile([128, 512], F32, tag="cosq")
                sinq_sb = pXq.tile([128, 512], F32, tag="sinq")
                for xc in range(0, 4, 2):
                    nc.sync.dma_start(
                        xq[:, xc * 512 : (xc + 2) * 512], xTq[:, xc * 512 : (xc + 2) * 512]
                    )
                nc.sync.dma_start(cosq_sb[:], cosq[:])
                nc.sync.dma_start(sinq_sb[:], sinq[:])
                for xc in range(4, 12, 2):
                    nc.sync.dma_start(
                        xq[:, xc * 512 : (xc + 2) * 512], xTq[:, xc * 512 : (xc + 2) * 512]
                    )
                wq_pre[16] = load_wq(16)
                for xc in range(12, 16, 2):
                    nc.sync.dma_start(
                        xq[:, xc * 512 : (xc + 2) * 512], xTq[:, xc * 512 : (xc + 2) * 512]
                    )
                for _t in range(1, N_WARMUP):
                    wq_pre[_t] = load_wq(_t)
                    wq_pre[16 + _t] = load_wq(16 + _t)

                def emit_qg_tile(t, ps_pool=None, ps_bufs=1):
                    """qg projection m-tile t (q-tile if t<16 else gate)."""
                    wq_t = wq_pre.pop(t) if t in wq_pre else load_wq(t)
                    qg_ps = (ps_pool or PSqg).tile(
                        [128, 512], F32, tag="qg", bufs=ps_bufs, name="qg_ps"
                    )
                    for kc in range(16):
                        nc.tensor.matmul(
                            qg_ps[:],
                            wq_t[:, kc * 128 : (kc + 1) * 128],
                            xq[:, kc * 512 : (kc + 1) * 512],
                            start=(kc == 0),
                            stop=(kc == 15),
                        )
                    if t < 16:
                        shf = pRt.tile([128, 512], F32, tag="shf", name="shf")
                        nc.vector.stream_shuffle(shf[:], qg_ps[:], _SHUF_MASK)
                        t1 = pRt.tile([128, 512], F32, tag="t1", name="t1")
                        nc.vector.tensor_tensor(t1[:], qg_ps[:], cosq_sb[:], ALU.mult)
                        t2 = pRt.tile([128, 512], F32, tag="t2", name="t2")
                        nc.vector.tensor_tensor(t2[:], shf[:], sinq_sb[:], ALU.mult)
                        nc.vector.tensor_tensor(
                            qT[:, t * 512 : (t + 1) * 512], t1[:], t2[:], ALU.add
                        )
                    else:
                        # e^{-g}; 1/(1+e^{-g}) is folded into the gating recip
                        nc.scalar.activation(
                            sigT[:, (t - 16) * 512 : (t - 15) * 512],
                            qg_ps[:],
                            AF.Exp,
                            scale=-1.0,
                        )

                # ---- phase A DMAs on the SCALAR queue so the warmup-stream
                # wq tile DMAs (pool-gated) can't head-of-line block them ----
                with (
                    tc.tile_pool(name="pAtab", bufs=1) as pAtab,
                    tc.tile_pool(name="pA", bufs=1) as pA,
                ):
                    wk_all = pA.tile([128, 16 * 512], BF16, tag="wk")
                    xk_all = pA.tile([128, 16 * 2048], BF16, tag="xk")
                    for kh in range(16):
                        nc.scalar.dma_start(
                            wk_all[:, kh * 512 : (kh + 1) * 512],
                            wkp[kh * 128 : (kh + 1) * 128, :],
                        )
                        nc.scalar.dma_start(
                            xk_all[:, kh * 2048 : (kh + 1) * 2048],
                            xTk[kh * 128 : (kh + 1) * 128, :],
                        )
                    cosk_sb = pAtab.tile([128, 2048], BF16, tag="cosk")
                    sink_sb = pAtab.tile([128, 2048], BF16, tag="sink")
                    nc.scalar.dma_start(cosk_sb[:], cosk[:])
                    nc.scalar.dma_start(sink_sb[:], sink[:])
                    nc.scalar.dma_start(mask_sb[:], maskd[:])

                    # ---- warmup: qg tiles run while wk/xk stream in. ----
                    with nc.named_scope("warmup"):
                        with tc.tile_pool(name="PSwarm", bufs=1, space="PSUM") as PSwarm:
                            for w in range(N_WARMUP):
                                emit_qg_tile(w, PSwarm, 2)
                                emit_qg_tile(16 + w, PSwarm, 2)

                    # ---- kproj (dense: xk fully resident by now) ----
                    with nc.named_scope("kproj"), (
                        tc.tile_pool(name="psA", bufs=1, space="PSUM")
                    ) as psA:
                        def emit_transposes(hk):
                            """kaug chunks for kv head hk from the RoPE'd kT.
                            Interleaved at tf boundaries so dense kproj MMs
                            surround the transpose-mode stretches (HAM does
                            not count transpose-mode as PE-busy)."""
                            hkr = (hk % 2) * 64
                            base = (hk // 2) * 2048
                            for jj in range(4):
                                tr = psA.tile([128, 256], BF16, tag="tr", bufs=2, name="tr")
                                for u in range(4):
                                    jc = jj * 4 + u
                                    nc.tensor.transpose(
                                        tr[:, u * 64 : (u + 1) * 64],
                                        kT[hkr : hkr + 64, base + jc * 128 : base + (jc + 1) * 128],
                                        ident[hkr : hkr + 64, :],
                                    )
                                nc.vector.tensor_copy(
                                    kaug4[:, hk, jj * 4 : (jj + 1) * 4, 0:64],
                                    tr[:].rearrange("p (u d) -> p u d", u=4),
                                )

                        for tf in range(4):
                            for kb in range(4):
                                kp_ps = psA.tile(
                                    [128, 512], F32, tag="kp", bufs=2, name="kp_ps"
                                )
                                for kc in range(16):
                                    nc.tensor.matmul(
                                        kp_ps[:],
                                        wk_all[:, kc * 512 + tf * 128 : kc * 512 + (tf + 1) * 128],
                                        xk_all[:, kc * 2048 + kb * 512 : kc * 2048 + (kb + 1) * 512],
                                        start=(kc == 0),
                                        stop=(kc == 15),
                                    )
                                shf = pRt.tile([128, 512], F32, tag="shf")
                                nc.vector.stream_shuffle(shf[:], kp_ps[:], _SHUF_MASK)
                                t1 = pRt.tile([128, 512], F32, tag="t1")
                                nc.vector.tensor_tensor(
                                    t1[:], kp_ps[:], cosk_sb[:, kb * 512 : (kb + 1) * 512], ALU.mult
                                )
                                t2 = pRt.tile([128, 512], F32, tag="t2")
                                nc.vector.tensor_tensor(
                                    t2[:], shf[:], sink_sb[:, kb * 512 : (kb + 1) * 512], ALU.mult
                                )
                                nc.vector.tensor_tensor(
                                    kT[:, tf * 2048 + kb * 512 : tf * 2048 + (kb + 1) * 512],
                                    t1[:],
                                    t2[:],
                                    ALU.add,
                                )
                            if tf >= 1 and tf < 3:
                                emit_transposes(2 * (tf - 1))
                                emit_transposes(2 * (tf - 1) + 1)
                        # dense qg work between the tail transpose stretches
                        # keeps HAM at 8/8 into attention (transpose-mode
                        # reads as PE-idle to the clock governor)
                        emit_transposes(4)
                        emit_qg_tile(N_WARMUP)
                        emit_transposes(5)
                        emit_qg_tile(16 + N_WARMUP)
                        emit_transposes(6)
                        emit_transposes(7)
                nc.vector.memset(kaug4[:, :, :, 64:65], 1.0)

                # ---- attention pair-steps ----
                with (
                    tc.tile_pool(name="pG", bufs=1) as pG,
                    tc.tile_pool(name="pO", bufs=3) as pO,
                ):
                    gatedT = pG.tile([128, 16 * 512], BF16, tag="gatedT")

                    wo_ts = {}
                    def load_wo(oc):
                        # gpsimd queue: idle during attention/outproj, so the
                        # ~0.6us-per-DMA dispatch cost never blocks the exp
                        # activations (scalar) or output copies
                        lst = []
                        for fc in range(16):
                            wo_t = pO.tile([128, 512], BF16, tag="wo", bufs=48, name="wo_t")
                            nc.gpsimd.dma_start(
                                wo_t[:],
                                wop[fc * 128 : (fc + 1) * 128, oc * 512 : (oc + 1) * 512],
                            )
                            lst.append(wo_t)
                        wo_ts[oc] = lst
                    # prefetch outproj wo during attention (scalar DMA queue
                    # is idle after kproj inputs land)
                    load_wo(0)
                    load_wo(1)

                    with nc.named_scope("attn"), (
                        tc.tile_pool(name="pET", bufs=5)
                    ) as pET, (
                        tc.tile_pool(name="pSm", bufs=1)
                    ) as pSm, (
                        tc.tile_pool(name="PSsc", bufs=1, space="PSUM")
                    ) as PSsc, (
                        tc.tile_pool(name="PSav", bufs=1, space="PSUM")
                    ) as PSav:
                        def sc_pair_group(kbase, jcs, w, rhs_A, rhs_B):
                          """Score matmuls + exp for one chunk group of BOTH
                          heads, MMs interleaved A/B so consecutive PE instrs
                          target opposite row-tiles (0,0)/(64,0) and their
                          LDWEIGHTS can pull ahead past the other tile's
                          in-flight matmul."""
                          scA = PSsc.tile([128, 1024], F32, tag="sc", bufs=2, name="scA")
                          scB = PSsc.tile([128, 1024], F32, tag="sc", bufs=2, name="scB")
                          for u, jc in enumerate(jcs):
                              nc.tensor.matmul(
                                  scA[:, u * w : u * w + w],
                                  kT[0:64, kbase + jc * 128 : kbase + (jc + 1) * 128],
                                  rhs_A,
                                  start=True,
                                  stop=True,
                              )
                              nc.tensor.matmul(
                                  scB[:, u * w : u * w + w],
                                  kT[64:128, kbase + jc * 128 : kbase + (jc + 1) * 128],
                                  rhs_B,
                                  start=True,
                                  stop=True,
                              )
                          n_tot = len(jcs) * w
                          etA = pET.tile([128, 1024], BF16, tag="et", name="etA")
                          nc.scalar.activation(etA[:, 0:n_tot], scA[:, 0:n_tot], AF.Exp)
                          etB = pET.tile([128, 1024], BF16, tag="et", name="etB")
                          nc.scalar.activation(etB[:, 0:n_tot], scB[:, 0:n_tot], AF.Exp)
                          return etA, etB

                        def masks(et, jcs, w, nja, njb):
                            # causal triangle masks: DVE multiply against the
                            # host-built constants (mask_sb[:,0:256] = q>=k,
                            # [:,256:512] = q>=k+128); keeps the strict-FIFO
                            # gpsimd queue out of the av critical path
                            for u, jc in enumerate(jcs):
                                if jc >= nja - 2 and jc < nja and w == 512:
                                    mi = 0 if jc == nja - 2 else 1
                                    nc.vector.tensor_tensor(
                                        et[:, u * w : u * w + 256],
                                        et[:, u * w : u * w + 256],
                                        mask_sb[:, mi * 256 : (mi + 1) * 256],
                                        ALU.mult,
                                    )
                                if jc >= njb - 2:
                                    off = u * w + (256 if w == 512 else 0)
                                    mi = 0 if jc == njb - 2 else 1
                                    nc.vector.tensor_tensor(
                                        et[:, off : off + 256],
                                        et[:, off : off + 256],
                                        mask_sb[:, mi * 256 : (mi + 1) * 256],
                                        ALU.mult,
                                    )

                        def av_group(av_ps, hk, jcs, w, et, njb):
                            for u, jc in enumerate(jcs):
                                nc.tensor.matmul(
                                    av_ps[0:65, (0 if w == 512 else 256) : 512],
                                    kaug[:, (hk * 16 + jc) * 65 : (hk * 16 + jc) * 65 + 65],
                                    et[:, u * w : u * w + w],
                                    start=(jc == 0),
                                    stop=(jc == njb - 1),
                                    skip_group_check=True,
                                )

                        def emit_gating(h, av_ps):
                            # gated = av / (l * (1 + e^{-g}))
                            tq, hr = _QTILE[h], _QROW[h]
                            lrow = pSm.tile([1, 512], F32, tag="lrow", name="lrow")
                            nc.vector.tensor_copy(lrow[:], av_ps[64:65, :])
                            lb = pSm.tile([64, 512], F32, tag="lb", name="lb")
                            nc.gpsimd.partition_broadcast(lb[:], lrow[:])
                            eg = sigT[hr : hr + 64, tq * 512 : (tq + 1) * 512]
                            if hr:
                                egc = pSm.tile([64, 512], BF16, tag="egc", name="egc")
                                nc.vector.tensor_copy(egc[:], eg)
                                eg = egc[:]
                            den = pSm.tile([64, 512], F32, tag="den", name="den")
                            nc.vector.scalar_tensor_tensor(
                                den[:], eg, 1.0, lb[:], ALU.add, ALU.mult
                            )
                            rden = pSm.tile([64, 512], F32, tag="rden", name="rden")
                            nc.vector.reciprocal_approx_fast(rden[:], den[:])
                            nc.vector.tensor_tensor(
                                gatedT[hr : hr + 64, tq * 512 : (tq + 1) * 512],
                                av_ps[0:64, :],
                                rden[:],
                                ALU.mult,
                            )

                        for arm in tc.Switch(ci, 4, hint=sw_hint):
                            nja, njb = 2 * arm + 2, 16 - 2 * arm
                            groups = []
                            for g0 in range(0, nja, 2):
                                groups.append((range(g0, min(g0 + 2, nja)), 512))
                            for g0 in range(nja, njb, 4):
                                groups.append((range(g0, min(g0 + 4, njb)), 256))

                            deferred = []
                            for t in range(16):
                                hA, hB = _EVENS[t], _ODDS[t]
                                hkA, hkB = hA // 4, hB // 4
                                kbase = (hkA // 2) * 2048
                                rhs_mA = qT[0:64, t * 512 : (t + 1) * 512]
                                rhs_sA = qT[0:64, t * 512 + 256 : (t + 1) * 512]
                                rhs_mB = qT[64:128, t * 512 : (t + 1) * 512]
                                rhs_sB = qT[64:128, t * 512 + 256 : (t + 1) * 512]
                                av_A = PSav.tile([65, 512], F32, tag="av", bufs=3, name="av_A")
                                av_B = PSav.tile([65, 512], F32, tag="av", bufs=3, name="av_B")

                                # one filler qg tile per pair (PE ~12us/pair
                                # vs scalar ~11.1us of exps — balanced):
                                # even pair 2k carries q-tile 8+k, odd pair
                                # 2k+1 carries gate-tile 24+k
                                filler = [(8 if t % 2 == 0 else 24) + t // 2]
                                if t == 0:
                                    # bridge pair-0's sc->exp pipeline
                                    # bootstrap with dense qg work
                                    emit_qg_tile(filler.pop(0))
                                pend = []
                                nflush = 0
                                for jcs, w in groups:
                                    etA, etB = sc_pair_group(
                                        kbase, jcs, w,
                                        rhs_mA if w == 512 else rhs_sA,
                                        rhs_mB if w == 512 else rhs_sB,
                                    )
                                    masks(etA, jcs, w, nja, njb)
                                    masks(etB, jcs, w, nja, njb)
                                    if deferred:
                                        # previous pair's gating lands on the
                                        # DVE queue only after this pair's
                                        # first masks, so the first av flush
                                        # never waits behind gating chains
                                        for h, av in deferred:
                                            emit_gating(h, av)
                                        deferred = []
                                    pend.append((jcs, w, etA, etB))
                                    if len(pend) > 1:
                                        jcs0, w0, eA, eB = pend.pop(0)
                                        av_group(av_A, hkA, jcs0, w0, eA, njb)
                                        av_group(av_B, hkB, jcs0, w0, eB, njb)
                                        nflush += 1
                                        if nflush == 1 and filler:
                                            emit_qg_tile(filler.pop(0))
                                while len(pend) > 1:
                                    jcs0, w0, eA, eB = pend.pop(0)
                                    av_group(av_A, hkA, jcs0, w0, eA, njb)
                                    av_group(av_B, hkB, jcs0, w0, eB, njb)
                                jcs0, w0, eA, eB = pend.pop(0)
                                av_group(av_A, hkA, jcs0, w0, eA, njb)
                                if t == 15:
                                    # last pair: gate A while B's final av
                                    # group runs so outproj isn't serialized
                                    # behind both gating chains
                                    emit_gating(hA, av_A)
                                    av_group(av_B, hkB, jcs0, w0, eB, njb)
                                    emit_gating(hB, av_B)
                                else:
                                    av_group(av_B, hkB, jcs0, w0, eB, njb)
                                    while filler:
                                        emit_qg_tile(filler.pop(0))
                                    deferred = [(hA, av_A), (hB, av_B)]
                            for h, av in deferred:
                                emit_gating(h, av)

                    # ---- out projection ----
                    with nc.named_scope("outproj"), (
                        tc.tile_pool(name="psO", bufs=1, space="PSUM")
                    ) as psO:
                        for oc in range(4):
                            if oc == 0:
                                # sets 2+3 stream during oc0/oc1 compute on
                                # the otherwise-idle scalar queue
                                load_wo(2)
                                load_wo(3)
                            for mi in range(4):
                                op_ps = psO.tile(
                                    [128, 512], F32, tag="op", bufs=3, name="op_ps"
                                )
                                for fc in range(16):
                                    nc.tensor.matmul(
                                        op_ps[:],
                                        gatedT[:, fc * 512 + mi * 128 : fc * 512 + (mi + 1) * 128],
                                        wo_ts[oc][fc][:],
                                        start=(fc == 0),
                                        stop=(fc == 15),
                                    )
                                o_sb = pO.tile([128, 512], F32, tag="ob", bufs=3, name="o_sb")
                                if oc == 3 and mi == 3:
                                    # split the tail copy+DMA so the final
                                    # write starts before the full copy ends
                                    for hh in range(2):
                                        nc.scalar.copy(
                                            o_sb[:, hh * 256 : (hh + 1) * 256],
                                            op_ps[:, hh * 256 : (hh + 1) * 256],
                                        )
                                        nc.sync.dma_start(
                                            out[
                                                mi * 128 : (mi + 1) * 128,
                                                oc * 512 + hh * 256 : oc * 512 + (hh + 1) * 256,
                                            ],
                                            o_sb[:, hh * 256 : (hh + 1) * 256],
                                        )
                                else:
                                    nc.scalar.copy(o_sb[:], op_ps[:])
                                    nc.sync.dma_start(
                                        out[mi * 128 : (mi + 1) * 128, oc * 512 : (oc + 1) * 512],
                                        o_sb[:],
                                    )
                            del wo_ts[oc]
    nc.compile()
    return nc


def _get_nc():
    global _NC_CACHE
    if _NC_CACHE is None:
        _NC_CACHE = _build_nc()
    return _NC_CACHE


def _prep_inputs(hidden_states, cos, sin, wq, wk, wo):
    """Build the 8 per-core input maps (all host-side slicing/permutation)."""
    inv = _INV
    dmap = np.concatenate([inv, inv])  # d index for partition p (p%64)
    sign = np.where((np.arange(128) % 64) % 2 == 0, -1.0, 1.0).astype(np.float32)

    wq_q = wq[:, :2048][:, _QCOL_ORDER]
    wq_g = wq[:, 2048:][:, _QCOL_ORDER]
    wqp_flat = np.concatenate([wq_q, wq_g], axis=1)  # [HID, 4096]
    wqp = np.ascontiguousarray(
        wqp_flat.reshape(16, 128, 32, 128).transpose(2, 1, 0, 3).reshape(32 * 128, 16 * 128)
    ).astype(ml_dtypes.bfloat16)
    wkp = np.ascontiguousarray(
        wk.reshape(HID, NKV, D)[:, :, inv].reshape(HID, 512)
    ).astype(ml_dtypes.bfloat16)
    wop = np.ascontiguousarray(wo[_QCOL_ORDER, :]).astype(ml_dtypes.bfloat16)

    # causal triangle masks for the diagonal 128x256 blocks:
    # maskd[:, 0:256][k, q] = 1 iff q >= k ; [:, 256:512][k, q] = 1 iff q >= k+128
    karr = np.arange(128)[:, None]
    qarr = np.arange(256)[None, :]
    maskd = np.concatenate(
        [(qarr >= karr), (qarr >= karr + 128)], axis=1
    ).astype(ml_dtypes.bfloat16)
    maskd = np.ascontiguousarray(maskd)

    in_maps = []
    for c in range(8):
        b, cc = c // 4, c % 4
        r0a, r0b = cc * 256, (7 - cc) * 256
        qrows = np.concatenate([np.arange(r0a, r0a + 256), np.arange(r0b, r0b + 256)])
        xT = hidden_states[b].T  # [HID, S]
        cq = cos[qrows][:, dmap].T  # [128, 512]
        sq = (sin[qrows][:, dmap].T * sign[:, None]).astype(np.float32)
        ck = cos[:, dmap].T  # [128, 2048] all key positions
        sk = (sin[:, dmap].T * sign[:, None]).astype(np.float32)
        in_maps.append(
            {
                "xTq": np.ascontiguousarray(
                    xT[:, qrows].reshape(16, 128, 512).transpose(1, 0, 2).reshape(128, 16 * 512)
                ).astype(ml_dtypes.bfloat16),
                "xTk": np.ascontiguousarray(xT).astype(ml_dtypes.bfloat16),
                "wqp": wqp,
                "wkp": wkp,
                "wop": wop,
                "cosq": np.ascontiguousarray(cq),
                "sinq": np.ascontiguousarray(sq),
                "cosk": np.ascontiguousarray(ck).astype(ml_dtypes.bfloat16),
                "sink": np.ascontiguousarray(sk).astype(ml_dtypes.bfloat16),
                "maskd": maskd,
            }
        )
    return in_maps


def kernel(hidden_states, cos, sin, attention_mask, wq, wk, wv, wo, **_unused):
    hidden_states = np.asarray(hidden_states, dtype=np.float32)
    cos = np.asarray(cos, dtype=np.float32)
    sin = np.asarray(sin, dtype=np.float32)
    wq = np.asarray(wq, dtype=np.float32)
    wk = np.asarray(wk, dtype=np.float32)
    wo = np.asarray(wo, dtype=np.float32)

    nc = _get_nc()
    in_maps = _prep_inputs(hidden_states, cos, sin, wq, wk, wo)
    res = run_bass_kernel_spmd(nc, in_maps, core_ids=list(range(8)))

    y = np.empty((B, S, HID), dtype=np.float32)
    for c in range(8):
        b, cc = c // 4, c % 4
        r0a, r0b = cc * 256, (7 - cc) * 256
        o = res.results[c]["out"]
        y[b, r0a : r0a + 256] = o[0:256]
        y[b, r0b : r0b + 256] = o[256:512]
    return y


# revision 20
# speedup vs baseline: 1.0529x; 1.0025x over previous
"""Trainium2 Bass kernel for nn_Attention_60739427500161.

Strategy (8 NeuronCores, one chip, no collectives):
- Sequence-sharded (context parallel): core c handles batch b=c//4 and two
  zigzag 256-row query strips (ci*256 and (7-ci)*256, ci=c%4) so causal work
  is balanced. Each core computes q/gate for its 512 rows, the full-batch k
  projection locally, runs attention + gating + out_proj for its rows, and
  writes its 512 output rows. The host scatters them back.
- All matmuls run in bf16. Scores are computed transposed (scoresT[j,i]) so
  softmax needs no PE transposes; denominator l via a ones-column in the av
  matmul; gating computes av / (l * (1 + e^{-g})).
- RoPE rotate_half is a host-side feature permutation so the partner lives
  one partition away and a DVE stream_shuffle([o^1]) produces the rotated
  operand.
- Phase layout: [warmup: 14 qg tiles while wk/xk DMA streams on the scalar
  queue] -> [kproj with transposes interleaved at tf boundaries, 2 qg tiles
  in the transpose tail] -> [16 attention pair-steps, pairs 0-7 carrying 2
  qg-filler tiles each] -> [outproj, wo prefetched during attention].
- Trace-driven scheduling facts baked in (see git/session notes):
  * Issue-to-issue MM gaps run at stream rate (216ns N=512, 109ns N=256,
    30ns transposes) — PE pipelining is fine; stalls are semaphore waits.
  * Causal masking via gpsimd affine_select (~1us each, strict FIFO queue)
    made diagonal av matmuls wait 0.8-1.4us per pair. Replaced with DVE
    tensor_tensor multiplies against two host-provided constant 0/1
    triangle masks (maskd[:, 0:256] = q>=k, [:, 256:512] = q>=k+128).
  * tc.Switch dispatch costs ~4us of PE-queue dead time (two serialized
    DRAM jump-table TENSOR_LOADs). tc.switch_hint at t=0 pre-stages the
    offset + instruction prefetch during the DMA lead-in shadow.
  * Warmup was DMA-bound with head-of-line stalls: wq tiles behind the xq
    bulk on the sync queue. Now: wq0 + tables first, xq in fine chunks,
    wq16 mid-stream, then eager wq pairs (pool-paced, bufs=4).
  * The 11.9us PE gap before kproj (xk tail wait) is filled by 2 extra
    warmup tiles; the transpose tail (HAM sees transpose-mode as idle)
    carries qg tiles 7/23 so the PE clock stays at 8/8 into attention.
  * wo loads used to trickle at 52GB/s during attention and stall outproj
    oc2/3 matmuls 0.5-1.7us each; now 2 sets prefetch at attention start,
    set 2 at pair 10, set 3 during outproj oc0 (scalar queue is idle then).
  * gpsimd ucode library (partition_broadcast) preloaded at t=0: first use
    otherwise costs a ~7us LIBRARY_RELOAD stall mid-pairs.
  * PSUM budget is exactly 8 banks: sc 2x[128,1024] + av 3x[65,512] + qg 1.
- wqp/xTq are host-pre-laid-out so every wq/xq DMA is a dense
  4-16KB-per-partition transfer.
- Chip-level P0 power-state variance is real: identical binaries measure
  +/-9% run to run (2.0 vs 2.4GHz PE clock under sustained load).
"""

import sys

for _p in ("/root/.axon_site/_ro/trn_rl_repo", "/opt/trn_rl_repo"):
    if _p not in sys.path:
        sys.path.append(_p)

import ml_dtypes
import numpy as np

import concourse.bass as bass
import concourse.mybir as mybir
import concourse.tile as tile
from concourse import bacc
from concourse.bass_utils import run_bass_kernel_spmd
from concourse.masks import make_identity

F32 = mybir.dt.float32
BF16 = mybir.dt.bfloat16
AF = mybir.ActivationFunctionType
ALU = mybir.AluOpType
ET = mybir.EngineType

B, S, HID = 2, 2048, 2048
NH, NKV, D = 32, 8, 64

# pi permutation: interleave (d, d+32) pairs so rotate_half partner is the
# adjacent partition. pos(d) = 2d (d<32) else 2(d-32)+1.
_POS = np.array([2 * d if d < 32 else 2 * (d - 32) + 1 for d in range(D)])
_INV = np.argsort(_POS)
_SHUF_MASK = [o ^ 1 for o in range(32)]

# q-head placement: head h must share its SBUF row base (0 or 64) with its
# kv head hk=h//4. Tile t pairs one even-hk head (rows 0-63) with one odd-hk
# head (rows 64-127); both heads of tile t form attention pair t.
_EVENS = [h for h in range(NH) if (h // 4) % 2 == 0]
_ODDS = [h for h in range(NH) if (h // 4) % 2 == 1]
_QTILE = [0] * NH
_QROW = [0] * NH
for _i, _h in enumerate(_EVENS):
    _QTILE[_h], _QROW[_h] = _i, 0
for _i, _h in enumerate(_ODDS):
    _QTILE[_h], _QROW[_h] = _i, 64
_QCOL_ORDER = np.concatenate(
    [np.concatenate([_EVENS[t] * D + _INV, _ODDS[t] * D + _INV]) for t in range(16)]
)

_NC_CACHE = None
N_WARMUP = 7  # qg tile-pairs (q+gate) emitted before kproj


def _build_nc():
    nc = bacc.Bacc(None, target_bir_lowering=False, enable_partition_id=True)

    xTq = nc.dram_tensor("xTq", [128, 16 * 512], BF16, kind="ExternalInput")
    xTk = nc.dram_tensor("xTk", [HID, S], BF16, kind="ExternalInput")
    wqp = nc.dram_tensor("wqp", [32 * 128, 16 * 128], BF16, kind="ExternalInput")
    wkp = nc.dram_tensor("wkp", [HID, 512], BF16, kind="ExternalInput")
    wop = nc.dram_tensor("wop", [HID, HID], BF16, kind="ExternalInput")
    cosq = nc.dram_tensor("cosq", [128, 512], F32, kind="ExternalInput")
    sinq = nc.dram_tensor("sinq", [128, 512], F32, kind="ExternalInput")
    cosk = nc.dram_tensor("cosk", [128, 2048], BF16, kind="ExternalInput")
    sink = nc.dram_tensor("sink", [128, 2048], BF16, kind="ExternalInput")
    maskd = nc.dram_tensor("maskd", [128, 512], BF16, kind="ExternalInput")
    out = nc.dram_tensor("out", [512, HID], F32, kind="ExternalOutput")

    with tile.TileContext(nc) as tc:
        ci = nc.partition_id() % 4
        # Pre-stage the attention Switch dispatch (offset regs + branch
        # prefetch) under the DMA lead-in shadow; the dispatch itself then
        # avoids ~4us of serialized DRAM jump-table loads on the PE queue.
        sw_hint = tc.switch_hint(
            index={e: ci for e in (ET.PE, ET.DVE, ET.Activation, ET.Pool, ET.SP)},
            n=4,
        )

        with tc.tile_pool(name="persist", bufs=1) as pers:
            qT = pers.tile([128, 16 * 512], BF16, tag="qT")
            sigT = pers.tile([128, 16 * 512], BF16, tag="sigT")
            kT = pers.tile([128, 4 * 2048], BF16, tag="kT")
            kaug = pers.tile([128, NKV * 16 * 65], BF16, tag="kaug")
            kaug4 = kaug[:].rearrange("p (h j d) -> p h j d", h=NKV, j=16)
            mask_sb = pers.tile([128, 512], BF16, tag="maskd")

            ident = pers.tile([128, 64], BF16, tag="ident")
            make_identity(nc, ident[0:64, :])
            nc.sync.dma_start(ident[64:128, :], ident[0:64, :])

            # Load the gpsimd partition_broadcast ucode library NOW, under
            # the DMA lead-in shadow: first use otherwise costs a ~7us
            # LIBRARY_RELOAD stall mid-pairs.
            gpw = pers.tile([2, 64], F32, tag="gpw")
            nc.vector.memset(gpw[:], 0.0)
            nc.gpsimd.partition_broadcast(gpw[:], gpw[0:1, :])

            with (
                tc.tile_pool(name="pXq", bufs=1) as pXq,
                tc.tile_pool(name="pWq", bufs=2) as pWq,
                tc.tile_pool(name="pRt", bufs=2) as pRt,
                tc.tile_pool(name="PSqg", bufs=1, space="PSUM") as PSqg,
            ):
                # ---- warmup-stream DMAs (sync queue) ----
                # Order: first tile's wq + RoPE tables, xq in fine chunks
                # (every qg tile contracts over ALL of xq, so xq lands as
                # early as possible), wq16 mid-stream, then eager wq pairs.
                # pWq bufs=4 paces the eager loads to consumption order.
                xq = pXq.tile([128, 16 * 512], BF16, tag="xq")

                def load_wq(t):
                    # host pre-layout (t, p, kc, m): one dense 4KB/partition
                    # transfer (the old (kc p) m gather read 256B segments)
                    wq_t = pWq.tile([128, 16 * 128], BF16, tag="wq", bufs=4, name="wq_t")
                    nc.sync.dma_start(wq_t[:], wqp[t * 128 : (t + 1) * 128, :])
                    return wq_t

                wq_pre = {0: load_wq(0)}
                cosq_sb = pXq.tile([128, 512], F32, tag="cosq")
                sinq_sb = pXq.tile([128, 512], F32, tag="sinq")
                for xc in range(0, 4, 2):
                    nc.sync.dma_start(
                        xq[:, xc * 512 : (xc + 2) * 512], xTq[:, xc * 512 : (xc + 2) * 512]
                    )
                nc.sync.dma_start(cosq_sb[:], cosq[:])
                nc.sync.dma_start(sinq_sb[:], sinq[:])
                for xc in range(4, 12, 2):
                    nc.sync.dma_start(
                        xq[:, xc * 512 : (xc + 2) * 512], xTq[:, xc * 512 : (xc + 2) * 512]
                    )
                wq_pre[16] = load_wq(16)
                for xc in range(12, 16, 2):
                    nc.sync.dma_start(
                        xq[:, xc * 512 : (xc + 2) * 512], xTq[:, xc * 512 : (xc + 2) * 512]
                    )
                for _t in range(1, N_WARMUP):
                    wq_pre[_t] = load_wq(_t)
                    wq_pre[16 + _t] = load_wq(16 + _t)

                def emit_qg_tile(t, ps_pool=None, ps_bufs=1):
                    """qg projection m-tile t (q-tile if t<16 else gate)."""
                    wq_t = wq_pre.pop(t) if t in wq_pre else load_wq(t)
                    qg_ps = (ps_pool or PSqg).tile(
                        [128, 512], F32, tag="qg", bufs=ps_bufs, name="qg_ps"
                    )
                    for kc in range(16):
                        nc.tensor.matmul(
                            qg_ps[:],
                            wq_t[:, kc * 128 : (kc + 1) * 128],
                            xq[:, kc * 512 : (kc + 1) * 512],
                            start=(kc == 0),
                            stop=(kc == 15),
                        )
                    if t < 16:
                        shf = pRt.tile([128, 512], F32, tag="shf", name="shf")
                        nc.vector.stream_shuffle(shf[:], qg_ps[:], _SHUF_MASK)
                        t1 = pRt.tile([128, 512], F32, tag="t1", name="t1")
                        nc.vector.tensor_tensor(t1[:], qg_ps[:], cosq_sb[:], ALU.mult)
                        t2 = pRt.tile([128, 512], F32, tag="t2", name="t2")
                        nc.vector.tensor_tensor(t2[:], shf[:], sinq_sb[:], ALU.mult)
                        nc.vector.tensor_tensor(
                            qT[:, t * 512 : (t + 1) * 512], t1[:], t2[:], ALU.add
                        )
                    else:
                        # e^{-g}; 1/(1+e^{-g}) is folded into the gating recip
                        nc.scalar.activation(
                            sigT[:, (t - 16) * 512 : (t - 15) * 512],
                            qg_ps[:],
                            AF.Exp,
                            scale=-1.0,
                        )

                # ---- phase A DMAs on the SCALAR queue so the warmup-stream
                # wq tile DMAs (pool-gated) can't head-of-line block them ----
                with (
                    tc.tile_pool(name="pAtab", bufs=1) as pAtab,
                    tc.tile_pool(name="pA", bufs=1) as pA,
                ):
                    wk_all = pA.tile([128, 16 * 512], BF16, tag="wk")
                    xk_all = pA.tile([128, 16 * 2048], BF16, tag="xk")
                    for kh in range(16):
                        nc.scalar.dma_start(
                            wk_all[:, kh * 512 : (kh + 1) * 512],
                            wkp[kh * 128 : (kh + 1) * 128, :],
                        )
                        nc.scalar.dma_start(
                            xk_all[:, kh * 2048 : (kh + 1) * 2048],
                            xTk[kh * 128 : (kh + 1) * 128, :],
                        )
                    cosk_sb = pAtab.tile([128, 2048], BF16, tag="cosk")
                    sink_sb = pAtab.tile([128, 2048], BF16, tag="sink")
                    nc.scalar.dma_start(cosk_sb[:], cosk[:])
                    nc.scalar.dma_start(sink_sb[:], sink[:])
                    nc.scalar.dma_start(mask_sb[:], maskd[:])

                    # ---- warmup: qg tiles run while wk/xk stream in. ----
                    with nc.named_scope("warmup"):
                        with tc.tile_pool(name="PSwarm", bufs=1, space="PSUM") as PSwarm:
                            for w in range(N_WARMUP):
                                emit_qg_tile(w, PSwarm, 2)
                                emit_qg_tile(16 + w, PSwarm, 2)

                    # ---- kproj (dense: xk fully resident by now) ----
                    with nc.named_scope("kproj"), (
                        tc.tile_pool(name="psA", bufs=1, space="PSUM")
                    ) as psA:
                        def emit_transposes(hk):
                            """kaug chunks for kv head hk from the RoPE'd kT.
                            Interleaved at tf boundaries so dense kproj MMs
                            surround the transpose-mode stretches (HAM does
                            not count transpose-mode as PE-busy)."""
                            hkr = (hk % 2) * 64
                            base = (hk // 2) * 2048
                            for jj in range(4):
                                tr = psA.tile([128, 256], BF16, tag="tr", bufs=2, name="tr")
                                for u in range(4):
                                    jc = jj * 4 + u
                                    nc.tensor.transpose(
                                        tr[:, u * 64 : (u + 1) * 64],
                                        kT[hkr : hkr + 64, base + jc * 128 : base + (jc + 1) * 128],
                                        ident[hkr : hkr + 64, :],
                                    )
                                nc.vector.tensor_copy(
                                    kaug4[:, hk, jj * 4 : (jj + 1) * 4, 0:64],
                                    tr[:].rearrange("p (u d) -> p u d", u=4),
                                )

                        for tf in range(4):
                            for kb in range(4):
                                kp_ps = psA.tile(
                                    [128, 512], F32, tag="kp", bufs=2, name="kp_ps"
                                )
                                for kc in range(16):
                                    nc.tensor.matmul(
                                        kp_ps[:],
                                        wk_all[:, kc * 512 + tf * 128 : kc * 512 + (tf + 1) * 128],
                                        xk_all[:, kc * 2048 + kb * 512 : kc * 2048 + (kb + 1) * 512],
                                        start=(kc == 0),
                                        stop=(kc == 15),
                                    )
                                shf = pRt.tile([128, 512], F32, tag="shf")
                                nc.vector.stream_shuffle(shf[:], kp_ps[:], _SHUF_MASK)
                                t1 = pRt.tile([128, 512], F32, tag="t1")
                                nc.vector.tensor_tensor(
                                    t1[:], kp_ps[:], cosk_sb[:, kb * 512 : (kb + 1) * 512], ALU.mult
                                )
                                t2 = pRt.tile([128, 512], F32, tag="t2")
                                nc.vector.tensor_tensor(
                                    t2[:], shf[:], sink_sb[:, kb * 512 : (kb + 1) * 512], ALU.mult
                                )
                                nc.vector.tensor_tensor(
                                    kT[:, tf * 2048 + kb * 512 : tf * 2048 + (kb + 1) * 512],
                                    t1[:],
                                    t2[:],
                                    ALU.add,
                                )
                            if tf >= 1 and tf < 3:
                                emit_transposes(2 * (tf - 1))
                                emit_transposes(2 * (tf - 1) + 1)
                        # dense qg work between the tail transpose stretches
                        # keeps HAM at 8/8 into attention (transpose-mode
                        # reads as PE-idle to the clock governor)
                        emit_transposes(4)
                        emit_qg_tile(N_WARMUP)
                        emit_transposes(5)
                        emit_qg_tile(16 + N_WARMUP)
                        emit_transposes(6)
                        emit_transposes(7)
                nc.vector.memset(kaug4[:, :, :, 64:65], 1.0)

                # ---- attention pair-steps ----
                with (
                    tc.tile_pool(name="pG", bufs=1) as pG,
                    tc.tile_pool(name="pO", bufs=3) as pO,
                ):
                    gatedT = pG.tile([128, 16 * 512], BF16, tag="gatedT")

                    wo_ts = {}
                    def load_wo(oc):
                        # gpsimd queue: idle during attention/outproj, so the
                        # ~0.6us-per-DMA dispatch cost never blocks the exp
                        # activations (scalar) or output copies
                        lst = []
                        for fc in range(16):
                            wo_t = pO.tile([128, 512], BF16, tag="wo", bufs=48, name="wo_t")
                            nc.gpsimd.dma_start(
                                wo_t[:],
                                wop[fc * 128 : (fc + 1) * 128, oc * 512 : (oc + 1) * 512],
                            )
                            lst.append(wo_t)
                        wo_ts[oc] = lst
                    # prefetch outproj wo during attention (scalar DMA queue
                    # is idle after kproj inputs land)
                    load_wo(0)
                    load_wo(1)

                    with nc.named_scope("attn"), (
                        tc.tile_pool(name="pET", bufs=5)
                    ) as pET, (
                        tc.tile_pool(name="pSm", bufs=1)
                    ) as pSm, (
                        tc.tile_pool(name="PSsc", bufs=1, space="PSUM")
                    ) as PSsc, (
                        tc.tile_pool(name="PSav", bufs=1, space="PSUM")
                    ) as PSav:
                        def sc_pair_group(kbase, jcs, w, rhs_A, rhs_B):
                          """Score matmuls + exp for one chunk group of BOTH
                          heads, MMs interleaved A/B so consecutive PE instrs
                          target opposite row-tiles (0,0)/(64,0) and their
                          LDWEIGHTS can pull ahead past the other tile's
                          in-flight matmul."""
                          scA = PSsc.tile([128, 1024], F32, tag="sc", bufs=2, name="scA")
                          scB = PSsc.tile([128, 1024], F32, tag="sc", bufs=2, name="scB")
                          for u, jc in enumerate(jcs):
                              nc.tensor.matmul(
                                  scA[:, u * w : u * w + w],
                                  kT[0:64, kbase + jc * 128 : kbase + (jc + 1) * 128],
                                  rhs_A,
                                  start=True,
                                  stop=True,
                              )
                              nc.tensor.matmul(
                                  scB[:, u * w : u * w + w],
                                  kT[64:128, kbase + jc * 128 : kbase + (jc + 1) * 128],
                                  rhs_B,
                                  start=True,
                                  stop=True,
                              )
                          n_tot = len(jcs) * w
                          etA = pET.tile([128, 1024], BF16, tag="et", name="etA")
                          nc.scalar.activation(etA[:, 0:n_tot], scA[:, 0:n_tot], AF.Exp)
                          etB = pET.tile([128, 1024], BF16, tag="et", name="etB")
                          nc.scalar.activation(etB[:, 0:n_tot], scB[:, 0:n_tot], AF.Exp)
                          return etA, etB

                        def masks(et, jcs, w, nja, njb):
                            # causal triangle masks: DVE multiply against the
                            # host-built constants (mask_sb[:,0:256] = q>=k,
                            # [:,256:512] = q>=k+128); keeps the strict-FIFO
                            # gpsimd queue out of the av critical path
                            for u, jc in enumerate(jcs):
                                if jc >= nja - 2 and jc < nja and w == 512:
                                    mi = 0 if jc == nja - 2 else 1
                                    nc.vector.tensor_tensor(
                                        et[:, u * w : u * w + 256],
                                        et[:, u * w : u * w + 256],
                                        mask_sb[:, mi * 256 : (mi + 1) * 256],
                                        ALU.mult,
                                    )
                                if jc >= njb - 2:
                                    off = u * w + (256 if w == 512 else 0)
                                    mi = 0 if jc == njb - 2 else 1
                                    nc.vector.tensor_tensor(
                                        et[:, off : off + 256],
                                        et[:, off : off + 256],
                                        mask_sb[:, mi * 256 : (mi + 1) * 256],
                                        ALU.mult,
                                    )

                        def av_group(av_ps, hk, jcs, w, et, njb):
                            for u, jc in enumerate(jcs):
                                nc.tensor.matmul(
                                    av_ps[0:65, (0 if w == 512 else 256) : 512],
                                    kaug[:, (hk * 16 + jc) * 65 : (hk * 16 + jc) * 65 + 65],
                                    et[:, u * w : u * w + w],
                                    start=(jc == 0),
                                    stop=(jc == njb - 1),
                                    skip_group_check=True,
                                )

                        def emit_gating(h, av_ps):
                            # gated = av / (l * (1 + e^{-g}))
                            tq, hr = _QTILE[h], _QROW[h]
                            lrow = pSm.tile([1, 512], F32, tag="lrow", name="lrow")
                            nc.vector.tensor_copy(lrow[:], av_ps[64:65, :])
                            lb = pSm.tile([64, 512], F32, tag="lb", name="lb")
                            nc.gpsimd.partition_broadcast(lb[:], lrow[:])
                            eg = sigT[hr : hr + 64, tq * 512 : (tq + 1) * 512]
                            if hr:
                                egc = pSm.tile([64, 512], BF16, tag="egc", name="egc")
                                nc.vector.tensor_copy(egc[:], eg)
                                eg = egc[:]
                            den = pSm.tile([64, 512], F32, tag="den", name="den")
                            nc.vector.scalar_tensor_tensor(
                                den[:], eg, 1.0, lb[:], ALU.add, ALU.mult
                            )
                            rden = pSm.tile([64, 512], F32, tag="rden", name="rden")
                            nc.vector.reciprocal_approx_fast(rden[:], den[:])
                            nc.vector.tensor_tensor(
                                gatedT[hr : hr + 64, tq * 512 : (tq + 1) * 512],
                                av_ps[0:64, :],
                                rden[:],
                                ALU.mult,
                            )

                        for arm in tc.Switch(ci, 4, hint=sw_hint):
                            nja, njb = 2 * arm + 2, 16 - 2 * arm
                            groups = []
                            for g0 in range(0, nja, 2):
                                groups.append((range(g0, min(g0 + 2, nja)), 512))
                            for g0 in range(nja, njb, 4):
                                groups.append((range(g0, min(g0 + 4, njb)), 256))

                            deferred = []
                            for t in range(16):
                                hA, hB = _EVENS[t], _ODDS[t]
                                hkA, hkB = hA // 4, hB // 4
                                kbase = (hkA // 2) * 2048
                                rhs_mA = qT[0:64, t * 512 : (t + 1) * 512]
                                rhs_sA = qT[0:64, t * 512 + 256 : (t + 1) * 512]
                                rhs_mB = qT[64:128, t * 512 : (t + 1) * 512]
                                rhs_sB = qT[64:128, t * 512 + 256 : (t + 1) * 512]
                                av_A = PSav.tile([65, 512], F32, tag="av", bufs=3, name="av_A")
                                av_B = PSav.tile([65, 512], F32, tag="av", bufs=3, name="av_B")

                                # one filler qg tile per pair (PE ~12us/pair
                                # vs scalar ~11.1us of exps — balanced):
                                # even pair 2k carries q-tile 8+k, odd pair
                                # 2k+1 carries gate-tile 24+k
                                filler = [(8 if t % 2 == 0 else 24) + t // 2]
                                if t == 0:
                                    # bridge pair-0's sc->exp pipeline
                                    # bootstrap with dense qg work
                                    emit_qg_tile(filler.pop(0))
                                pend = []
                                nflush = 0
                                for jcs, w in groups:
                                    etA, etB = sc_pair_group(
                                        kbase, jcs, w,
                                        rhs_mA if w == 512 else rhs_sA,
                                        rhs_mB if w == 512 else rhs_sB,
                                    )
                                    masks(etA, jcs, w, nja, njb)
                                    masks(etB, jcs, w, nja, njb)
                                    if deferred:
                                        # previous pair's gating lands on the
                                        # DVE queue only after this pair's
                                        # first masks, so the first av flush
                                        # never waits behind gating chains
                                        for h, av in deferred:
                                            emit_gating(h, av)
                                        deferred = []
                                    pend.append((jcs, w, etA, etB))
                                    if len(pend) > 1:
                                        jcs0, w0, eA, eB = pend.pop(0)
                                        av_group(av_A, hkA, jcs0, w0, eA, njb)
                                        av_group(av_B, hkB, jcs0, w0, eB, njb)
                                        nflush += 1
                                        if nflush == 1 and filler:
                                            emit_qg_tile(filler.pop(0))
                                while len(pend) > 1:
                                    jcs0, w0, eA, eB = pend.pop(0)
                                    av_group(av_A, hkA, jcs0, w0, eA, njb)
                                    av_group(av_B, hkB, jcs0, w0, eB, njb)
                                jcs0, w0, eA, eB = pend.pop(0)
                                av_group(av_A, hkA, jcs0, w0, eA, njb)
                                if t == 15:
                                    # last pair: gate A while B's final av
                                    # group runs so outproj isn't serialized
                                    # behind both gating chains
                                    emit_gating(hA, av_A)
                                    av_group(av_B, hkB, jcs0, w0, eB, njb)
                                    emit_gating(hB, av_B)
                                else:
                                    av_group(av_B, hkB, jcs0, w0, eB, njb)
                                    while filler:
                                        emit_qg_tile(filler.pop(0))
                                    deferred = [(hA, av_A), (hB, av_B)]
                            for h, av in deferred:
                                emit_gating(h, av)

                    # ---- out projection ----
                    with nc.named_scope("outproj"), (
                        tc.tile_pool(name="psO", bufs=1, space="PSUM")
                    ) as psO:
                        for oc in range(4):
                            if oc == 0:
                                # sets 2+3 stream during oc0/oc1 compute on
                                # the otherwise-idle scalar queue
                                load_wo(2)
                                load_wo(3)
                            for mi in range(4):
                                op_ps = psO.tile(
                                    [128, 512], F32, tag="op", bufs=3, name="op_ps"
                                )
                                for fc in range(16):
                                    nc.tensor.matmul(
                                        op_ps[:],
                                        gatedT[:, fc * 512 + mi * 128 : fc * 512 + (mi + 1) * 128],
                                        wo_ts[oc][fc][:],
                                        start=(fc == 0),
                                        stop=(fc == 15),
                                    )
                                o_sb = pO.tile([128, 512], F32, tag="ob", bufs=3, name="o_sb")
                                if oc == 3 and mi == 3:
                                    # split the tail copy+DMA so the final
                                    # write starts before the full copy ends
                                    for hh in range(2):
                                        nc.scalar.copy(
                                            o_sb[:, hh * 256 : (hh + 1) * 256],
                                            op_ps[:, hh * 256 : (hh + 1) * 256],
                                        )
                                        nc.sync.dma_start(
                                            out[
                                                mi * 128 : (mi + 1) * 128,
                                                oc * 512 + hh * 256 : oc * 512 + (hh + 1) * 256,
                                            ],
                                            o_sb[:, hh * 256 : (hh + 1) * 256],
                                        )
                                else:
                                    nc.scalar.copy(o_sb[:], op_ps[:])
                                    nc.sync.dma_start(
                                        out[mi * 128 : (mi + 1) * 128, oc * 512 : (oc + 1) * 512],
                                        o_sb[:],
                                    )
                            del wo_ts[oc]
    nc.compile()
    return nc


def _get_nc():
    global _NC_CACHE
    if _NC_CACHE is None:
        _NC_CACHE = _build_nc()
    return _NC_CACHE


def _prep_inputs(hidden_states, cos, sin, wq, wk, wo):
    """Build the 8 per-core input maps (all host-side slicing/permutation)."""
    inv = _INV
    dmap = np.concatenate([inv, inv])  # d index for partition p (p%64)
    sign = np.where((np.arange(128) % 64) % 2 == 0, -1.0, 1.0).astype(np.float32)

    wq_q = wq[:, :2048][:, _QCOL_ORDER]
    wq_g = wq[:, 2048:][:, _QCOL_ORDER]
    wqp_flat = np.concatenate([wq_q, wq_g], axis=1)  # [HID, 4096]
    wqp = np.ascontiguousarray(
        wqp_flat.reshape(16, 128, 32, 128).transpose(2, 1, 0, 3).reshape(32 * 128, 16 * 128)
    ).astype(ml_dtypes.bfloat16)
    wkp = np.ascontiguousarray(
        wk.reshape(HID, NKV, D)[:, :, inv].reshape(HID, 512)
    ).astype(ml_dtypes.bfloat16)
    wop = np.ascontiguousarray(wo[_QCOL_ORDER, :]).astype(ml_dtypes.bfloat16)

    # causal triangle masks for the diagonal 128x256 blocks:
    # maskd[:, 0:256][k, q] = 1 iff q >= k ; [:, 256:512][k, q] = 1 iff q >= k+128
    karr = np.arange(128)[:, None]
    qarr = np.arange(256)[None, :]
    maskd = np.concatenate(
        [(qarr >= karr), (qarr >= karr + 128)], axis=1
    ).astype(ml_dtypes.bfloat16)
    maskd = np.ascontiguousarray(maskd)

    in_maps = []
    for c in range(8):
        b, cc = c // 4, c % 4
        r0a, r0b = cc * 256, (7 - cc) * 256
        qrows = np.concatenate([np.arange(r0a, r0a + 256), np.arange(r0b, r0b + 256)])
        xT = hidden_states[b].T  # [HID, S]
        cq = cos[qrows][:, dmap].T  # [128, 512]
        sq = (sin[qrows][:, dmap].T * sign[:, None]).astype(np.float32)
        ck = cos[:, dmap].T  # [128, 2048] all key positions
        sk = (sin[:, dmap].T * sign[:, None]).astype(np.float32)
        in_maps.append(
            {
                "xTq": np.ascontiguousarray(
                    xT[:, qrows].reshape(16, 128, 512).transpose(1, 0, 2).reshape(128, 16 * 512)
                ).astype(ml_dtypes.bfloat16),
                "xTk": np.ascontiguousarray(xT).astype(ml_dtypes.bfloat16),
                "wqp": wqp,
                "wkp": wkp,
                "wop": wop,
                "cosq": np.ascontiguousarray(cq),
                "sinq": np.ascontiguousarray(sq),
                "cosk": np.ascontiguousarray(ck).astype(ml_dtypes.bfloat16),
                "sink": np.ascontiguousarray(sk).astype(ml_dtypes.bfloat16),
                "maskd": maskd,
            }
        )
    return in_maps


def kernel(hidden_states, cos, sin, attention_mask, wq, wk, wv, wo, **_unused):
    hidden_states = np.asarray(hidden_states, dtype=np.float32)
    cos = np.asarray(cos, dtype=np.float32)
    sin = np.asarray(sin, dtype=np.float32)
    wq = np.asarray(wq, dtype=np.float32)
    wk = np.asarray(wk, dtype=np.float32)
    wo = np.asarray(wo, dtype=np.float32)

    nc = _get_nc()
    in_maps = _prep_inputs(hidden_states, cos, sin, wq, wk, wo)
    res = run_bass_kernel_spmd(nc, in_maps, core_ids=list(range(8)))

    y = np.empty((B, S, HID), dtype=np.float32)
    for c in range(8):
        b, cc = c // 4, c % 4
        r0a, r0b = cc * 256, (7 - cc) * 256
        o = res.results[c]["out"]
        y[b, r0a : r0a + 256] = o[0:256]
        y[b, r0b : r0b + 256] = o[256:512]
    return y
